# revision 1
# baseline (speedup 1.0000x reference)
"""CARAFE (content-aware upsample, power-normalized softmax) on 8 TRN2 cores.

Math (reference.py): X (2,256,64,64) ->
  conv1x1(256->64) + bn + relu -> conv3x3(64->100) + bn -> pixel_shuffle(2)
  -> W (2,25,128,128) -> softmax(clip(W)^p) over 25 taps
  out[b,c,y,x] = sum_{ki,kj} W[b,(ki,kj),y,x] * Xpad[b,c,y//2+ki-2,x//2+kj-2]

Strategy (pure data-parallel over h, 8 rows / core):
  * conv1x1 / conv3x3 as bf16 GEMMs (channels on partitions).
  * softmax via ACT transcendentals; tap-sum via a 100x4 selection matmul;
    reciprocal on a reshaped [128,16] tile.
  * The per-pixel 25-tap weighted sum is a banded matmul per output row h:
    out[c,(ry,x)] = sum_{p} XT_r[p,c] * B_ki[p,(ry,x)] accumulated over ki,
    where B_ki[p, col] = Wnorm[(ki,kj),...] iff p = x//2 + kj.  B is produced
    by writing the dense softmax output to a zero-padded DRAM scratch
    ("epad", donated-zero output buffer) and reading it back with a
    fused-stride DMA access pattern that materializes the banded layout
    (including the zeros) directly in SBUF.
  * XT_r strips come from PE transposes of the input rows.

kernel(**inputs) takes the FULL inputs and returns the FULL output.
"""

import numpy as np
import ml_dtypes

SCALE = 2
K_UP = 5
B, C, H, W = 2, 256, 64, 64
N_CORES = 8
HS = H // N_CORES            # 8 low-res rows per core
XROWS = HS + 4               # 12 rows (with +-2 halo)
WP = W + 4                   # 68 (w padded by 2 each side)
CMID, CENC = 64, 100
# epad: per (b,h,ki) a [64 w-blocks x 131 slots x 2 ry x 2 rx] zero-padded
# buffer; valid kj slots are 63..67 (slot = kj + ZOFF, kj = p - w).
NSLOT = 131
ZOFF = 63
SWB = 4 * NSLOT              # 524 elements per w-block
EPN = W * SWB                # 33536 elements per (b,h,ki)

_STATE = {}


def _build_nc():
    import concourse.bass as bass
    import concourse.tile as tile
    from concourse import mybir
    from concourse.vector_clock import ScopedClock
    from concourse.tile_rust import add_dep_helper

    # --- workaround: this walrus build rejects >1 sync-wait on CTRL-class
    # instructions; split the Tile tail-drain waits into 1-wait NOPs. ---
    def patched_drain_and_barrier(self, tick_clock, wait_clock):
        maxw = 1
        carrier = self.nc.sync.nop()
        wait_clock.add_sem_waits(
            carrier.ins, ScopedClock({None: tick_clock.global_clock})
        )
        si = carrier.ins.sync_info
        waits = list(si.on_wait) if si is not None else []
        if len(waits) > maxw:
            si.on_wait = waits[:maxw]
            carrier.ins.sync_info = si
            rest = waits[maxw:]
            for i in range(0, len(rest), maxw):
                n = self.nc.sync.nop()
                n.ins.sync_info = mybir.SyncInfo(
                    on_wait=rest[i : i + maxw], on_update=[]
                )
        self.nc.sync.drain()
        self.nc.all_engine_barrier()
        assert self.sems is not None
        popped = self.nc._tile_sem_poison_stack.pop()
        assert popped is self._sem_poison
        self.nc.clear_and_free_semaphores(list(self.sems.allocated().values()))
        self.nc.all_engine_barrier()

    tile.TileContext._drain_and_barrier = patched_drain_and_barrier

    # --- workaround #2: the same walrus build accepts at most ONE sync wait
    # on ANY instruction.  Post-process the serialized BIR: hoist excess
    # waits onto single-wait NoOps inserted just before, on the same engine
    # (same program point, so semantics are unchanged). ---
    import orjson

    def _split_waits_json(raw: bytes) -> bytes:
        j = orjson.loads(raw)
        n = 0
        changed = False
        for fn in j["functions"]:
            for bb in fn["blocks"]:
                out = []
                for ins in bb["instructions"]:
                    si = ins.get("sync_info")
                    waits = si.get("on_wait") if si else None
                    if waits and len(waits) > 1:
                        changed = True
                        for wt in waits[:-1]:
                            n += 1
                            out.append(
                                {
                                    "debug": ins.get("debug", 0),
                                    "engine": ins["engine"],
                                    "ins": [],
                                    "outs": [],
                                    "name": f"WSPL-{n}",
                                    "opcode": "NoOp",
                                    "sync_info": {"on_update": [], "on_wait": [wt]},
                                }
                            )
                        si["on_wait"] = [waits[-1]]
                    out.append(ins)
                bb["instructions"] = out
        return orjson.dumps(j) if changed else raw

    if not getattr(bass.Bass.to_json_bytes, "_wait_split", False):
        _orig_tjb = bass.Bass.to_json_bytes

        def patched_to_json_bytes(self):
            return _split_waits_json(_orig_tjb(self))

        patched_to_json_bytes._wait_split = True
        bass.Bass.to_json_bytes = patched_to_json_bytes

    f32 = mybir.dt.float32
    bf16 = mybir.dt.bfloat16
    AF = mybir.ActivationFunctionType

    nc = bass.Bass()

    # ---- parameters ----
    xh = nc.declare_dram_parameter("xh", [B, C, XROWS, WP], bf16, isOutput=False)
    comp_w = nc.declare_dram_parameter("comp_w", [CMID, C, 1, 1], f32, isOutput=False)
    c_g = nc.declare_dram_parameter("comp_gamma", [CMID], f32, isOutput=False)
    c_b = nc.declare_dram_parameter("comp_beta", [CMID], f32, isOutput=False)
    c_m = nc.declare_dram_parameter("comp_mean", [CMID], f32, isOutput=False)
    c_v = nc.declare_dram_parameter("comp_var", [CMID], f32, isOutput=False)
    enc_w = nc.declare_dram_parameter("enc_w", [CENC, CMID, 3, 3], f32, isOutput=False)
    e_g = nc.declare_dram_parameter("enc_gamma", [CENC], f32, isOutput=False)
    e_b = nc.declare_dram_parameter("enc_beta", [CENC], f32, isOutput=False)
    e_m = nc.declare_dram_parameter("enc_mean", [CENC], f32, isOutput=False)
    e_v = nc.declare_dram_parameter("enc_var", [CENC], f32, isOutput=False)
    p_in = nc.declare_dram_parameter("power_p", [1], f32, isOutput=False)
    sel = nc.declare_dram_parameter("sel", [CENC, 4], f32, isOutput=False)
    ident = nc.declare_dram_parameter("ident", [128, 128], bf16, isOutput=False)
    y1mask = nc.declare_dram_parameter("y1mask", [660], bf16, isOutput=False)

    out = nc.declare_dram_parameter(
        "out", [B, C, 2 * HS, 2 * W], f32, isOutput=True
    )
    # donated-zero scratch outputs (never read host-side)
    epad = nc.declare_dram_parameter("epad", [B, HS, K_UP, EPN], bf16, isOutput=True)
    rscr = nc.declare_dram_parameter("rscr", [B, 2048], f32, isOutput=True)
    pscr = nc.declare_dram_parameter("pscr", [1], f32, isOutput=True)

    def dram_ap(param, offset, dims):
        return bass.AP(tensor=param, offset=offset, ap=[list(d) for d in dims])

    with tile.TileContext(nc) as tc:
        import contextlib

        ctx = contextlib.ExitStack()
        const = ctx.enter_context(tc.tile_pool(name="const", bufs=1))
        stage = ctx.enter_context(tc.tile_pool(name="stage", bufs=2))
        sm = ctx.enter_context(tc.tile_pool(name="sm", bufs=2))
        xtp = ctx.enter_context(tc.tile_pool(name="xtp", bufs=24))
        bp = ctx.enter_context(tc.tile_pool(name="bp", bufs=12))
        op = ctx.enter_context(tc.tile_pool(name="op", bufs=4))
        ps_c1 = ctx.enter_context(tc.tile_pool(name="ps_c1", bufs=2, space="PSUM"))
        ps_c3 = ctx.enter_context(tc.tile_pool(name="ps_c3", bufs=1, space="PSUM"))
        ps_s = ctx.enter_context(tc.tile_pool(name="ps_s", bufs=1, space="PSUM"))
        ps_t = ctx.enter_context(tc.tile_pool(name="ps_t", bufs=2, space="PSUM"))
        ps_e = ctx.enter_context(tc.tile_pool(name="ps_e", bufs=2, space="PSUM"))

        # ---- constants in SBUF ----
        ident_sb = const.tile([128, 128], bf16, tag="ident")
        nc.sync.dma_start(out=ident_sb[:, :], in_=ident[:, :])
        sel_sb = const.tile([CENC, 4], f32, tag="sel")
        nc.sync.dma_start(out=sel_sb[:, :], in_=sel[:, :])
        mask_sb = const.tile([CMID, 10, 66], bf16, tag="mask")
        nc.sync.dma_start(
            out=mask_sb[:, :, :],
            in_=dram_ap(y1mask, 0, [[0, CMID], [66, 10], [1, 66]]),
        )

        # conv1x1 weights: lhsT [cin(128) x cout(64)] per cin-half
        comp_bf = []
        for ct in range(2):
            cf = stage.tile([128, CMID], f32, tag="wstage")
            nc.sync.dma_start(
                out=cf[:, :],
                in_=dram_ap(comp_w, ct * 128, [[1, 128], [C, CMID]]),
            )
            cb = const.tile([128, CMID], bf16, tag=f"comp_bf{ct}")
            nc.vector.tensor_copy(cb[:, :], cf[:, :])
            comp_bf.append(cb)

        # conv3x3 weights: lhsT [cin(64) x cout(100)] per (dy,dx)
        enc_bf = []
        for j in range(9):
            ef = stage.tile([CMID, CENC], f32, tag="wstage")
            nc.sync.dma_start(
                out=ef[:, :],
                in_=dram_ap(enc_w, j, [[9, CMID], [9 * CMID, CENC]]),
            )
            eb = const.tile([CMID, CENC], bf16, tag=f"enc_bf{j}")
            nc.vector.tensor_copy(eb[:, :], ef[:, :])
            enc_bf.append(eb)

        # ---- batchnorm fold: inv = gamma/sqrt(var+eps), shift = beta-mean*inv
        def bn_fold(gamma, beta, mean, var, n, tagp):
            g = const.tile([n, 1], f32, tag=f"{tagp}g")
            bt = const.tile([n, 1], f32, tag=f"{tagp}b")
            m = const.tile([n, 1], f32, tag=f"{tagp}m")
            v = const.tile([n, 1], f32, tag=f"{tagp}v")
            for t, src in ((g, gamma), (bt, beta), (m, mean), (v, var)):
                nc.sync.dma_start(out=t[:, :], in_=dram_ap(src, 0, [[1, n]]))
            eps = const.tile([n, 1], f32, tag=f"{tagp}e")
            nc.vector.memset(eps[:, :], 1e-5)
            std = const.tile([n, 1], f32, tag=f"{tagp}s")
            nc.scalar.activation(std[:, :], v[:, :], AF.Sqrt, bias=eps[:, :])
            rstd = const.tile([n, 1], f32, tag=f"{tagp}r")
            nc.vector.reciprocal(rstd[:, :], std[:, :])
            inv = const.tile([n, 1], f32, tag=f"{tagp}i")
            nc.vector.tensor_mul(inv[:, :], g[:, :], rstd[:, :])
            tmp = const.tile([n, 1], f32, tag=f"{tagp}t")
            nc.vector.tensor_mul(tmp[:, :], m[:, :], inv[:, :])
            shift = const.tile([n, 1], f32, tag=f"{tagp}h")
            nc.vector.tensor_sub(shift[:, :], bt[:, :], tmp[:, :])
            return inv, shift

        inv1, shift1 = bn_fold(c_g, c_b, c_m, c_v, CMID, "bn1")
        inv2, shift2 = bn_fold(e_g, e_b, e_m, e_v, CENC, "bn2")

        # ---- p = clip(power_p, 1e-5), broadcast to [100,1] via DRAM bounce
        p_sb = const.tile([1, 1], f32, tag="p")
        nc.sync.dma_start(out=p_sb[:, :], in_=dram_ap(p_in, 0, [[1, 1]]))
        nc.vector.tensor_scalar_max(p_sb[:, :], p_sb[:, :], 1e-5)
        p_wr = nc.sync.dma_start(out=dram_ap(pscr, 0, [[1, 1]]), in_=p_sb[:, :])
        pb_sb = const.tile([CENC, 1], f32, tag="pb")
        p_rd = nc.sync.dma_start(
            out=pb_sb[:, :], in_=dram_ap(pscr, 0, [[0, CENC], [1, 1]])
        )
        add_dep_helper(p_rd.ins, p_wr.ins, sync=True, reason="pscr RAW")

        # ---- X rows in SBUF (bf16, already padded on host) ----
        xbf = [[None, None] for _ in range(B)]
        for b in range(B):
            for ct in range(2):
                t = const.tile([128, XROWS, WP], bf16, tag=f"xbf{b}{ct}")
                nc.sync.dma_start(
                    out=t[:, :, :],
                    in_=dram_ap(
                        xh,
                        b * C * XROWS * WP + ct * 128 * XROWS * WP,
                        [[XROWS * WP, 128], [1, XROWS * WP]],
                    ),
                )
                xbf[b][ct] = t

        # ---- Y1 tiles (zeroed once; borders stay zero) ----
        y1 = []
        for b in range(B):
            t = const.tile([CMID, 10, 66], bf16, tag=f"y1_{b}")
            nc.vector.memset(t[:, :, :], 0.0)
            y1.append(t)

        epad_wr = {}  # (b,h) -> dma inst
        xts = {}  # (b,j) -> X^T strip tile

        for b in range(B):
            # ===== conv1x1 + bn1 + relu =====
            for half in range(2):
                pc = ps_c1.tile([CMID, 320], f32, tag="c1")
                for ct in range(2):
                    nc.tensor.matmul(
                        pc[:, :],
                        comp_bf[ct][:, :],
                        xbf[b][ct][:, 1 + 5 * half : 6 + 5 * half, 2 : 2 + W],
                        start=(ct == 0),
                        stop=(ct == 1),
                    )
                nc.scalar.activation(
                    y1[b][:, 5 * half : 5 * half + 5, 1 : 1 + W],
                    pc[:, :],
                    AF.Relu,
                    bias=shift1[:, :],
                    scale=inv1[:, :],
                )
            # zero out-of-image rows / padding cols
            nc.vector.tensor_mul(y1[b][:, :, :], y1[b][:, :, :], mask_sb[:, :, :])

            # ===== conv3x3 + bn2 =====
            pc3 = ps_c3.tile([CENC, HS * W], f32, tag="c3")
            jj = 0
            for dy in (-1, 0, 1):
                for dx in (-1, 0, 1):
                    nc.tensor.matmul(
                        pc3[:, :],
                        enc_bf[jj][:, :],
                        y1[b][:, 1 + dy : 9 + dy, 1 + dx : 1 + dx + W],
                        start=(jj == 0),
                        stop=(jj == 8),
                    )
                    jj += 1
            w_sb = sm.tile([CENC, HS * W], f32, tag="w")
            nc.scalar.activation(
                w_sb[:, :], pc3[:, :], AF.Identity, bias=shift2[:, :], scale=inv2[:, :]
            )

            # ===== power + softmax numerator =====
            nc.vector.tensor_scalar_max(w_sb[:, :], w_sb[:, :], 1e-5)
            nc.scalar.activation(w_sb[:, :], w_sb[:, :], AF.Ln)
            nc.scalar.activation(w_sb[:, :], w_sb[:, :], AF.Exp, scale=pb_sb[:, :])
            e_sb = sm.tile([CENC, HS * W], f32, tag="e")
            nc.scalar.activation(e_sb[:, :], w_sb[:, :], AF.Exp)

            # ===== tap-sums, reciprocal, normalize =====
            ps = ps_s.tile([4, HS * W], f32, tag="s")
            nc.tensor.matmul(ps[:, :], sel_sb[:, :], e_sb[:, :], start=True, stop=True)
            sums_sb = sm.tile([4, 32, 16], f32, tag="sums")
            nc.vector.tensor_copy(sums_sb[:, :, :], ps[:, :].rearrange("p (a b) -> p a b", b=16))
            s128 = sm.tile([128, 16], f32, tag="s128")
            nc.sync.dma_start(out=s128[:, :], in_=sums_sb[:, :, :])
            r128 = sm.tile([128, 16], f32, tag="r128")
            nc.vector.reciprocal(r128[:, :], s128[:, :])
            r_wr = nc.sync.dma_start(
                out=dram_ap(rscr, b * 2048, [[16, 128], [1, 16]]), in_=r128[:, :]
            )
            rb_sb = sm.tile([CENC, HS * W], f32, tag="rb")
            r_rd = nc.sync.dma_start(
                out=rb_sb[:, :],
                in_=dram_ap(rscr, b * 2048, [[0, 25], [512, 4], [1, 512]]),
            )
            add_dep_helper(r_rd.ins, r_wr.ins, sync=True, reason="rscr RAW")
            en_sb = sm.tile([CENC, HS, W], bf16, tag="en")
            nc.vector.tensor_mul(
                en_sb[:, :, :],
                e_sb[:, :].rearrange("p (a b) -> p a b", b=W),
                rb_sb[:, :].rearrange("p (a b) -> p a b", b=W),
            )

            # ===== scatter normalized weights to banded DRAM scratch =====
            # per ki: src [20 partitions, 64]; dst slots (kj,ry,rx) are the
            # 20 contiguous elements at ZOFF*4, strided SWB per w-block.
            for h in range(HS):
                base = (b * HS + h) * K_UP * EPN
                wrs = []
                for ki in range(K_UP):
                    wr = nc.sync.dma_start(
                        out=dram_ap(
                            epad,
                            base + ki * EPN + ZOFF * 4,
                            [[1, 20], [SWB, W]],
                        ),
                        in_=en_sb[20 * ki : 20 * (ki + 1), h, :],
                    )
                    wrs.append(wr)
                epad_wr[(b, h)] = wrs

            # ===== X^T strips via PE transpose =====
            for j in range(XROWS):
                pt = ps_t.tile([WP, 256], bf16, tag="pt")
                for ct in range(2):
                    nc.tensor.transpose(
                        pt[:, ct * 128 : (ct + 1) * 128],
                        xbf[b][ct][:, j, :],
                        ident_sb[:, :],
                    )
                xt = xtp.tile([WP, 256], bf16, tag="xt")
                nc.any.tensor_copy(xt[:, :], pt[:, :])
                xts[(b, j)] = xt

            # ===== banded einsum =====
            for h in range(HS):
                bts = []
                for ki in range(K_UP):
                    bt = bp.tile([WP, 256], bf16, tag="bt")
                    for ry in range(2):
                        rd = nc.sync.dma_start(
                            out=bt[:, ry * 128 : (ry + 1) * 128],
                            in_=dram_ap(
                                epad,
                                (b * HS + h) * K_UP * EPN
                                + ki * EPN
                                + ZOFF * 4
                                + 2 * ry,
                                [[4, WP], [SWB - 4, W], [1, 2]],
                            ),
                        )
                        add_dep_helper(
                            rd.ins,
                            epad_wr[(b, h)][ki].ins,
                            sync=True,
                            reason="epad RAW",
                        )
                    bts.append(bt)
                for ct in range(2):
                    pe = ps_e.tile([128, 256], f32, tag="pe")
                    for ki in range(K_UP):
                        nc.tensor.matmul(
                            pe[:, :],
                            xts[(b, h + ki)][:, ct * 128 : (ct + 1) * 128],
                            bts[ki][:, :],
                            start=(ki == 0),
                            stop=(ki == K_UP - 1),
                        )
                    o_sb = op.tile([128, 256], f32, tag="osb")
                    nc.any.tensor_copy(o_sb[:, :], pe[:, :])
                    nc.sync.dma_start(
                        out=dram_ap(
                            out,
                            b * C * 2 * HS * 2 * W
                            + ct * 128 * 2 * HS * 2 * W
                            + 2 * h * 2 * W,
                            [[2 * HS * 2 * W, 128], [2 * W, 2], [1, 2 * W]],
                        ),
                        in_=o_sb[:, :],
                    )

        ctx.close()

    return nc


def _get_nc():
    if "nc" not in _STATE:
        _STATE["nc"] = _build_nc()
    return _STATE["nc"]


def _make_in_maps(inputs):
    X = np.asarray(inputs["X"], dtype=np.float32)
    Xp = np.pad(X, ((0, 0), (0, 0), (2, 2), (2, 2)))
    sel = np.zeros((CENC, 4), np.float32)
    for p in range(CENC):
        sel[p, p % 4] = 1.0
    ident = np.eye(128, dtype=ml_dtypes.bfloat16)
    common = {
        "comp_w": np.asarray(inputs["comp_w"], np.float32),
        "comp_gamma": np.asarray(inputs["comp_gamma"], np.float32),
        "comp_beta": np.asarray(inputs["comp_beta"], np.float32),
        "comp_mean": np.asarray(inputs["comp_mean"], np.float32),
        "comp_var": np.asarray(inputs["comp_var"], np.float32),
        "enc_w": np.asarray(inputs["enc_w"], np.float32),
        "enc_gamma": np.asarray(inputs["enc_gamma"], np.float32),
        "enc_beta": np.asarray(inputs["enc_beta"], np.float32),
        "enc_mean": np.asarray(inputs["enc_mean"], np.float32),
        "enc_var": np.asarray(inputs["enc_var"], np.float32),
        "power_p": np.asarray(inputs["power_p"], np.float32),
        "sel": sel,
        "ident": ident,
    }
    in_maps = []
    for core in range(N_CORES):
        r0 = HS * core
        xh = np.ascontiguousarray(Xp[:, :, r0 : r0 + XROWS, :]).astype(
            ml_dtypes.bfloat16
        )
        mask = np.zeros((10, 66), np.float32)
        for rr in range(10):
            grow = r0 - 1 + rr
            if 0 <= grow < H:
                mask[rr, 1 : 1 + W] = 1.0
        m = dict(common)
        m["xh"] = xh
        m["y1mask"] = mask.reshape(660).astype(ml_dtypes.bfloat16)
        in_maps.append(m)
    return in_maps


def _run(inputs, trace=False):
    from concourse.bass_utils import run_bass_kernel_spmd

    if trace:
        import sys, os
        sys.path.insert(0, os.path.dirname(os.path.abspath(__file__)))
        import hookshim  # noqa: F401

    nc = _get_nc()
    in_maps = _make_in_maps(inputs)
    res = run_bass_kernel_spmd(
        nc, in_maps, core_ids=list(range(N_CORES)), trace=trace
    )
    out = np.concatenate([res.results[c]["out"] for c in range(N_CORES)], axis=2)
    return out, res


def kernel(**inputs):
    out, _ = _run(inputs, trace=False)
    return out



# revision 9
# speedup vs baseline: 11.9185x; 11.9185x over previous
"""CARAFE (content-aware upsample, power-normalized softmax) on 8 TRN2 cores.

Math (reference.py): X (2,256,64,64) ->
  conv1x1(256->64) + bn + relu -> conv3x3(64->100) + bn -> pixel_shuffle(2)
  -> W (2,25,128,128) -> softmax(clip(W)^p) over 25 taps
  out[b,c,y,x] = sum_{ki,kj} W[b,(ki,kj),y,x] * Xpad[b,c,y//2+ki-2,x//2+kj-2]

Strategy (pure data-parallel over h, 8 low-res rows / core):
  * conv1x1 / conv3x3 as bf16 GEMMs (channels on partitions).
  * softmax via ACT transcendentals; tap-sums via a 100x4 selection matmul;
    reciprocal on [4,512]; denominator broadcast back to 100 partitions via
    a 4x100 selection matmul (no DRAM bounce).
  * The per-pixel 25-tap weighted sum is a banded matmul per output row h:
    out[c,(ry,x)] = sum_p XT_r[p,c] * B_ki[p,(ry,x)] accumulated over ki,
    where B_ki[w+kj, ry*128+2w+rx] = Wnorm[(ki,kj,ry,rx), h, w].  B is built
    ON-CHIP: PE-transpose Wnorm rows -> 5 partition-shift matmuls (constant
    shift matrices) -> one GPSIMD local_scatter per (b,h) placing the
    diagonal bands (per-partition indices, zeros implicit).
  * XT_r strips come from PE transposes of the input rows.

kernel(**inputs) takes the FULL inputs and returns the FULL output.
"""

import numpy as np
import ml_dtypes

SCALE = 2
K_UP = 5
B, C, H, W = 2, 256, 64, 64
N_CORES = 8
HS = H // N_CORES            # 8 low-res rows per core
XROWS = HS + 4               # 12 rows (with +-2 halo)
WP = W + 4                   # 68 (w padded by 2 each side)
CMID, CENC = 64, 100
NSLOT = K_UP * 2 * W * SCALE // 2  # bts columns per ki = 256
BTN = K_UP * 256             # 1280 elems per bts row
LCH = 80                     # local_scatter channels (68 rounded up to 16x)

_STATE = {}


def _build_nc():
    import concourse.bass as bass
    import concourse.tile as tile
    from concourse import mybir
    from concourse.vector_clock import ScopedClock
    from concourse.tile_rust import add_dep_helper

    # --- workaround: this walrus build rejects >1 sync-wait on CTRL-class
    # instructions; split the Tile tail-drain waits into 1-wait NOPs. ---
    def patched_drain_and_barrier(self, tick_clock, wait_clock):
        maxw = 1
        carrier = self.nc.sync.nop()
        wait_clock.add_sem_waits(
            carrier.ins, ScopedClock({None: tick_clock.global_clock})
        )
        si = carrier.ins.sync_info
        waits = list(si.on_wait) if si is not None else []
        if len(waits) > maxw:
            si.on_wait = waits[:maxw]
            carrier.ins.sync_info = si
            rest = waits[maxw:]
            for i in range(0, len(rest), maxw):
                n = self.nc.sync.nop()
                n.ins.sync_info = mybir.SyncInfo(
                    on_wait=rest[i : i + maxw], on_update=[]
                )
        self.nc.sync.drain()
        self.nc.all_engine_barrier()
        assert self.sems is not None
        popped = self.nc._tile_sem_poison_stack.pop()
        assert popped is self._sem_poison
        self.nc.clear_and_free_semaphores(list(self.sems.allocated().values()))
        self.nc.all_engine_barrier()

    tile.TileContext._drain_and_barrier = patched_drain_and_barrier

    # --- workaround #2: the same walrus build accepts at most ONE sync wait
    # on ANY instruction.  Post-process the serialized BIR: hoist excess
    # waits onto single-wait NoOps inserted just before, on the same engine
    # (same program point, so semantics are unchanged). ---
    import orjson

    def _split_waits_json(raw: bytes) -> bytes:
        j = orjson.loads(raw)
        n = 0
        changed = False
        for fn in j["functions"]:
            for bb in fn["blocks"]:
                out = []
                for ins in bb["instructions"]:
                    si = ins.get("sync_info")
                    waits = si.get("on_wait") if si else None
                    if waits and len(waits) > 1:
                        changed = True
                        for wt in waits[:-1]:
                            n += 1
                            out.append(
                                {
                                    "debug": ins.get("debug", 0),
                                    "engine": ins["engine"],
                                    "ins": [],
                                    "outs": [],
                                    "name": f"WSPL-{n}",
                                    "opcode": "NoOp",
                                    "sync_info": {"on_update": [], "on_wait": [wt]},
                                }
                            )
                        si["on_wait"] = [waits[-1]]
                    out.append(ins)
                bb["instructions"] = out
        return orjson.dumps(j) if changed else raw

    if not getattr(bass.Bass.to_json_bytes, "_wait_split", False):
        _orig_tjb = bass.Bass.to_json_bytes

        def patched_to_json_bytes(self):
            return _split_waits_json(_orig_tjb(self))

        patched_to_json_bytes._wait_split = True
        bass.Bass.to_json_bytes = patched_to_json_bytes

    f32 = mybir.dt.float32
    bf16 = mybir.dt.bfloat16
    i16 = mybir.dt.int16
    AF = mybir.ActivationFunctionType

    nc = bass.Bass()

    # ---- parameters ----
    xh = nc.declare_dram_parameter("xh", [B, C, XROWS, WP], bf16, isOutput=False)
    comp_w = nc.declare_dram_parameter("comp_w", [CMID, C, 1, 1], f32, isOutput=False)
    c_g = nc.declare_dram_parameter("comp_gamma", [CMID], f32, isOutput=False)
    c_b = nc.declare_dram_parameter("comp_beta", [CMID], f32, isOutput=False)
    c_m = nc.declare_dram_parameter("comp_mean", [CMID], f32, isOutput=False)
    c_v = nc.declare_dram_parameter("comp_var", [CMID], f32, isOutput=False)
    enc_w = nc.declare_dram_parameter("enc_w", [CENC, CMID, 3, 3], f32, isOutput=False)
    e_g = nc.declare_dram_parameter("enc_gamma", [CENC], f32, isOutput=False)
    e_b = nc.declare_dram_parameter("enc_beta", [CENC], f32, isOutput=False)
    e_m = nc.declare_dram_parameter("enc_mean", [CENC], f32, isOutput=False)
    e_v = nc.declare_dram_parameter("enc_var", [CENC], f32, isOutput=False)
    p_in = nc.declare_dram_parameter("power_p", [1], f32, isOutput=False)
    sel = nc.declare_dram_parameter("sel", [CENC, 4], f32, isOutput=False)
    selT = nc.declare_dram_parameter("selT", [4, CENC], f32, isOutput=False)
    ident = nc.declare_dram_parameter("ident", [128, 128], bf16, isOutput=False)
    y1mask = nc.declare_dram_parameter("y1mask", [660], bf16, isOutput=False)
    shmat = nc.declare_dram_parameter("shmat", [K_UP, W, LCH], bf16, isOutput=False)
    lsidx = nc.declare_dram_parameter("lsidx", [LCH, CENC], i16, isOutput=False)

    out = nc.declare_dram_parameter(
        "out", [B, C, 2 * HS, 2 * W], f32, isOutput=True
    )
    # donated-zero scratch output (never read host-side)
    pscr = nc.declare_dram_parameter("pscr", [1], f32, isOutput=True)

    def dram_ap(param, offset, dims):
        return bass.AP(tensor=param, offset=offset, ap=[list(d) for d in dims])

    with tile.TileContext(nc) as tc:
        import contextlib

        ctx = contextlib.ExitStack()
        const = ctx.enter_context(tc.tile_pool(name="const", bufs=1))
        stage = ctx.enter_context(tc.tile_pool(name="stage", bufs=2))
        sm = ctx.enter_context(tc.tile_pool(name="sm", bufs=2))
        xtp = ctx.enter_context(tc.tile_pool(name="xtp", bufs=24))
        dp = ctx.enter_context(tc.tile_pool(name="dp", bufs=4))
        btp = ctx.enter_context(tc.tile_pool(name="btp", bufs=10))
        op = ctx.enter_context(tc.tile_pool(name="op", bufs=4))
        ps_big = ctx.enter_context(tc.tile_pool(name="ps_big", bufs=2, space="PSUM"))
        ps_bf = ctx.enter_context(tc.tile_pool(name="ps_bf", bufs=2, space="PSUM"))
        ps_sh = ctx.enter_context(tc.tile_pool(name="ps_sh", bufs=2, space="PSUM"))
        ps_e = ctx.enter_context(tc.tile_pool(name="ps_e", bufs=2, space="PSUM"))

        # ---- constants in SBUF ----
        ident_sb = const.tile([128, 128], bf16, tag="ident")
        nc.sync.dma_start(out=ident_sb[:, :], in_=ident[:, :])
        sel_sb = const.tile([CENC, 4], f32, tag="sel")
        nc.sync.dma_start(out=sel_sb[:, :], in_=sel[:, :])
        selT_sb = const.tile([4, CENC], f32, tag="selT")
        nc.sync.dma_start(out=selT_sb[:, :], in_=selT[:, :])
        mask_sb = const.tile([CMID, 10, 66], bf16, tag="mask")
        nc.sync.dma_start(
            out=mask_sb[:, :, :],
            in_=dram_ap(y1mask, 0, [[0, CMID], [66, 10], [1, 66]]),
        )
        sh_all = const.tile([W, K_UP, LCH], bf16, tag="shmat")
        nc.sync.dma_start(
            out=sh_all[:, :, :],
            in_=dram_ap(shmat, 0, [[LCH, W], [W * LCH, K_UP], [1, LCH]]),
        )
        lsidx_sb = const.tile([LCH, CENC], i16, tag="lsidx")
        nc.sync.dma_start(out=lsidx_sb[:, :], in_=lsidx[:, :])

        # conv1x1 weights: lhsT [cin(128) x cout(64)] per cin-half
        comp_bf = []
        for ct in range(2):
            cf = stage.tile([128, CMID], f32, tag="wstage")
            nc.sync.dma_start(
                out=cf[:, :],
                in_=dram_ap(comp_w, ct * 128, [[1, 128], [C, CMID]]),
            )
            cb = const.tile([128, CMID], bf16, tag=f"comp_bf{ct}")
            nc.vector.tensor_copy(cb[:, :], cf[:, :])
            comp_bf.append(cb)

        # conv3x3 weights: lhsT [cin(64) x cout(100)] per (dy,dx)
        enc_bf = []
        for j in range(9):
            ef = stage.tile([CMID, CENC], f32, tag="wstage")
            nc.sync.dma_start(
                out=ef[:, :],
                in_=dram_ap(enc_w, j, [[9, CMID], [9 * CMID, CENC]]),
            )
            eb = const.tile([CMID, CENC], bf16, tag=f"enc_bf{j}")
            nc.vector.tensor_copy(eb[:, :], ef[:, :])
            enc_bf.append(eb)

        # ---- batchnorm fold: inv = gamma/sqrt(var+eps), shift = beta-mean*inv
        def bn_fold(gamma, beta, mean, var, n, tagp):
            g = const.tile([n, 1], f32, tag=f"{tagp}g")
            bt = const.tile([n, 1], f32, tag=f"{tagp}b")
            m = const.tile([n, 1], f32, tag=f"{tagp}m")
            v = const.tile([n, 1], f32, tag=f"{tagp}v")
            for t, src in ((g, gamma), (bt, beta), (m, mean), (v, var)):
                nc.sync.dma_start(out=t[:, :], in_=dram_ap(src, 0, [[1, n]]))
            eps = const.tile([n, 1], f32, tag=f"{tagp}e")
            nc.vector.memset(eps[:, :], 1e-5)
            std = const.tile([n, 1], f32, tag=f"{tagp}s")
            nc.scalar.activation(std[:, :], v[:, :], AF.Sqrt, bias=eps[:, :])
            rstd = const.tile([n, 1], f32, tag=f"{tagp}r")
            nc.vector.reciprocal(rstd[:, :], std[:, :])
            inv = const.tile([n, 1], f32, tag=f"{tagp}i")
            nc.vector.tensor_mul(inv[:, :], g[:, :], rstd[:, :])
            tmp = const.tile([n, 1], f32, tag=f"{tagp}t")
            nc.vector.tensor_mul(tmp[:, :], m[:, :], inv[:, :])
            shift = const.tile([n, 1], f32, tag=f"{tagp}h")
            nc.vector.tensor_sub(shift[:, :], bt[:, :], tmp[:, :])
            return inv, shift

        inv1, shift1 = bn_fold(c_g, c_b, c_m, c_v, CMID, "bn1")
        inv2, shift2 = bn_fold(e_g, e_b, e_m, e_v, CENC, "bn2")

        # ---- p = clip(power_p, 1e-5), broadcast to [100,1] via DRAM bounce
        p_sb = const.tile([1, 1], f32, tag="p")
        nc.sync.dma_start(out=p_sb[:, :], in_=dram_ap(p_in, 0, [[1, 1]]))
        nc.vector.tensor_scalar_max(p_sb[:, :], p_sb[:, :], 1e-5)
        p_wr = nc.sync.dma_start(out=dram_ap(pscr, 0, [[1, 1]]), in_=p_sb[:, :])
        pb_sb = const.tile([CENC, 1], f32, tag="pb")
        p_rd = nc.sync.dma_start(
            out=pb_sb[:, :], in_=dram_ap(pscr, 0, [[0, CENC], [1, 1]])
        )
        add_dep_helper(p_rd.ins, p_wr.ins, sync=True, reason="pscr RAW")

        # ---- X rows in SBUF (bf16, already padded on host) ----
        xbf = [[None, None] for _ in range(B)]
        for b in range(B):
            for ct in range(2):
                t = const.tile([128, XROWS, WP], bf16, tag=f"xbf{b}{ct}")
                nc.sync.dma_start(
                    out=t[:, :, :],
                    in_=dram_ap(
                        xh,
                        b * C * XROWS * WP + ct * 128 * XROWS * WP,
                        [[XROWS * WP, 128], [1, XROWS * WP]],
                    ),
                )
                xbf[b][ct] = t

        # ---- Y1 tiles (zeroed once; borders stay zero) ----
        y1 = []
        for b in range(B):
            t = const.tile([CMID, 10, 66], bf16, tag=f"y1_{b}")
            nc.vector.memset(t[:, :, :], 0.0)
            y1.append(t)

        xts = {}  # (b,j) -> X^T strip tile

        for b in range(B):
            # ===== conv1x1 + bn1 + relu =====
            for half in range(2):
                pcb = ps_big.tile([CENC, HS * W], f32, tag="big")
                pc = pcb[0:CMID, 0:320]
                for ct in range(2):
                    nc.tensor.matmul(
                        pc,
                        comp_bf[ct][:, :],
                        xbf[b][ct][:, 1 + 5 * half : 6 + 5 * half, 2 : 2 + W],
                        start=(ct == 0),
                        stop=(ct == 1),
                    )
                nc.scalar.activation(
                    y1[b][:, 5 * half : 5 * half + 5, 1 : 1 + W],
                    pc,
                    AF.Relu,
                    bias=shift1[:, :],
                    scale=inv1[:, :],
                )
            # zero out-of-image rows / padding cols
            nc.vector.tensor_mul(y1[b][:, :, :], y1[b][:, :, :], mask_sb[:, :, :])

            # ===== conv3x3 + bn2 =====
            pc3 = ps_big.tile([CENC, HS * W], f32, tag="big")
            jj = 0
            for dy in (-1, 0, 1):
                for dx in (-1, 0, 1):
                    nc.tensor.matmul(
                        pc3[:, :],
                        enc_bf[jj][:, :],
                        y1[b][:, 1 + dy : 9 + dy, 1 + dx : 1 + dx + W],
                        start=(jj == 0),
                        stop=(jj == 8),
                    )
                    jj += 1
            w_sb = sm.tile([CENC, HS * W], f32, tag="w")
            nc.scalar.activation(
                w_sb[:, :], pc3[:, :], AF.Identity, bias=shift2[:, :], scale=inv2[:, :]
            )

            # ===== power + softmax numerator =====
            nc.vector.tensor_scalar_max(w_sb[:, :], w_sb[:, :], 1e-5)
            nc.scalar.activation(w_sb[:, :], w_sb[:, :], AF.Ln)
            nc.scalar.activation(w_sb[:, :], w_sb[:, :], AF.Exp, scale=pb_sb[:, :])
            e_sb = sm.tile([CENC, HS * W], f32, tag="e")
            nc.scalar.activation(e_sb[:, :], w_sb[:, :], AF.Exp)

            # ===== tap-sums, reciprocal, broadcast, normalize =====
            psb = ps_big.tile([CENC, HS * W], f32, tag="big")
            ps = psb[0:4, :]
            nc.tensor.matmul(ps, sel_sb[:, :], e_sb[:, :], start=True, stop=True)
            r4_sb = sm.tile([4, HS * W], f32, tag="r4")
            nc.vector.reciprocal(r4_sb[:, :], ps)
            rb_ps = ps_big.tile([CENC, HS * W], f32, tag="big")
            nc.tensor.matmul(
                rb_ps[:, :], selT_sb[:, :], r4_sb[:, :], start=True, stop=True
            )
            en_sb = const.tile([CENC, HS, W], bf16, tag=f"en{b}")
            nc.vector.tensor_mul(
                en_sb[:, :, :],
                e_sb[:, :].rearrange("p (a b) -> p a b", b=W),
                rb_ps[:, :].rearrange("p (a b) -> p a b", b=W),
            )

            # ===== X^T strips via PE transpose =====
            for j in range(XROWS):
                pt = ps_bf.tile([WP, 256], bf16, tag="bf")
                for ct in range(2):
                    nc.tensor.transpose(
                        pt[:, ct * 128 : (ct + 1) * 128],
                        xbf[b][ct][:, j, :],
                        ident_sb[:, :],
                    )
                xt = xtp.tile([WP, 256], bf16, tag="xt")
                nc.scalar.activation(xt[:, :], pt[:, :], AF.Identity)
                xts[(b, j)] = xt

            # ===== banded-matrix build: transpose + shifts + local_scatter ===
            bts_all = []
            tp_sbs = {}

            def emit_transpose(h):
                tpt = ps_bf.tile([WP, 256], bf16, tag="bf")
                tp_ps = tpt[0:W, 0:CENC]
                nc.tensor.transpose(
                    tp_ps, en_sb[:, h, :], ident_sb[:CENC, :CENC]
                )
                t = dp.tile([W, CENC], bf16, tag="tps")
                nc.vector.tensor_copy(t[:, :], tp_ps)
                tp_sbs[h] = t

            emit_transpose(0)
            for h in range(HS):
                if h + 1 < HS:
                    emit_transpose(h + 1)
                tp_sb = tp_sbs.pop(h)
                sh_ps = ps_sh.tile([LCH, CENC], f32, tag="sh")
                tp_v = tp_sb[:, :].rearrange("p (a b) -> p a b", b=20)
                for s in range(K_UP):
                    nc.tensor.matmul(
                        sh_ps[:, 20 * s : 20 * (s + 1)],
                        sh_all[:, s, :],
                        tp_v[:, :, 4 * s : 4 * (s + 1)],
                        start=True,
                        stop=True,
                    )
                data_sb = dp.tile([LCH, CENC], bf16, tag="data")
                nc.vector.tensor_copy(data_sb[:, :], sh_ps[:, :])
                bts = btp.tile([LCH, BTN], bf16, tag="bts")
                nc.gpsimd.local_scatter(
                    out_ap=bts[:, :],
                    data_ap=data_sb[:, :],
                    idxs_ap=lsidx_sb[:, :],
                    channels=LCH,
                    num_elems=BTN,
                    num_idxs=CENC,
                )
                bts_all.append(bts)

            # ===== banded einsum =====
            for h in range(HS):
                bts = bts_all[h]
                for ct in range(2):
                    pe = ps_e.tile([128, 256], f32, tag="pe")
                    for ki in range(K_UP):
                        nc.tensor.matmul(
                            pe[:, :],
                            xts[(b, h + ki)][:, ct * 128 : (ct + 1) * 128],
                            bts[0:WP, ki * 256 : (ki + 1) * 256],
                            start=(ki == 0),
                            stop=(ki == K_UP - 1),
                        )
                    o_sb = op.tile([128, 256], f32, tag="osb")
                    nc.scalar.activation(o_sb[:, :], pe[:, :], AF.Identity)
                    nc.sync.dma_start(
                        out=dram_ap(
                            out,
                            b * C * 2 * HS * 2 * W
                            + ct * 128 * 2 * HS * 2 * W
                            + 2 * h * 2 * W,
                            [[2 * HS * 2 * W, 128], [2 * W, 2], [1, 2 * W]],
                        ),
                        in_=o_sb[:, :],
                    )

        ctx.close()

    # ---- Bacc-style finishing passes: library loads + ISA assembly ----
    from concourse.library_config import all_libraries, standard
    import bass_rust as _bass_rust

    lib_mask = {}
    for lib in all_libraries:
        for it in lib.instructions:
            lib_mask[it] = lib_mask.get(it, 0) | (1 << lib.index)
    _bass_rust.insert_library_loads(nc, lib_mask, len(all_libraries), standard.index)
    mybir.codegen_inst_isa_subclasses(nc)

    return nc


def _get_nc():
    if "nc" not in _STATE:
        _STATE["nc"] = _build_nc()
    return _STATE["nc"]


def _make_in_maps(inputs):
    X = np.asarray(inputs["X"], dtype=np.float32)
    Xp = np.pad(X, ((0, 0), (0, 0), (2, 2), (2, 2)))
    sel = np.zeros((CENC, 4), np.float32)
    for p in range(CENC):
        sel[p, p % 4] = 1.0
    ident = np.eye(128, dtype=ml_dtypes.bfloat16)
    shmat = np.zeros((K_UP, W, LCH), np.float32)
    for s in range(K_UP):
        for w in range(W):
            shmat[s, w, w + s] = 1.0
    lsidx = np.full((LCH, CENC), -1, np.int16)
    for p in range(WP):
        for s in range(K_UP):
            w = p - s
            if 0 <= w < W:
                for ki in range(K_UP):
                    for u in range(4):
                        ry, rx = u // 2, u % 2
                        c = s * 20 + ki * 4 + u
                        lsidx[p, c] = ki * 256 + ry * 128 + 2 * w + rx
    common = {
        "comp_w": np.asarray(inputs["comp_w"], np.float32),
        "comp_gamma": np.asarray(inputs["comp_gamma"], np.float32),
        "comp_beta": np.asarray(inputs["comp_beta"], np.float32),
        "comp_mean": np.asarray(inputs["comp_mean"], np.float32),
        "comp_var": np.asarray(inputs["comp_var"], np.float32),
        "enc_w": np.asarray(inputs["enc_w"], np.float32),
        "enc_gamma": np.asarray(inputs["enc_gamma"], np.float32),
        "enc_beta": np.asarray(inputs["enc_beta"], np.float32),
        "enc_mean": np.asarray(inputs["enc_mean"], np.float32),
        "enc_var": np.asarray(inputs["enc_var"], np.float32),
        "power_p": np.asarray(inputs["power_p"], np.float32),
        "sel": sel,
        "selT": sel.T.copy(),
        "ident": ident,
        "shmat": shmat.astype(ml_dtypes.bfloat16),
        "lsidx": lsidx,
    }
    in_maps = []
    for core in range(N_CORES):
        r0 = HS * core
        xh = np.ascontiguousarray(Xp[:, :, r0 : r0 + XROWS, :]).astype(
            ml_dtypes.bfloat16
        )
        mask = np.zeros((10, 66), np.float32)
        for rr in range(10):
            grow = r0 - 1 + rr
            if 0 <= grow < H:
                mask[rr, 1 : 1 + W] = 1.0
        m = dict(common)
        m["xh"] = xh
        m["y1mask"] = mask.reshape(660).astype(ml_dtypes.bfloat16)
        in_maps.append(m)
    return in_maps


def _run(inputs, trace=False):
    from concourse.bass_utils import run_bass_kernel_spmd

    if trace:
        import sys, os
        sys.path.insert(0, os.path.dirname(os.path.abspath(__file__)))
        import hookshim  # noqa: F401

    nc = _get_nc()
    in_maps = _make_in_maps(inputs)
    res = run_bass_kernel_spmd(
        nc, in_maps, core_ids=list(range(N_CORES)), trace=trace
    )
    out = np.concatenate([res.results[c]["out"] for c in range(N_CORES)], axis=2)
    return out, res


def kernel(**inputs):
    out, _ = _run(inputs, trace=False)
    return out


# revision 25
# speedup vs baseline: 16.6837x; 1.3998x over previous
"""CARAFE (content-aware upsample, power-normalized softmax) on 8 TRN2 cores.

Math (reference.py): X (2,256,64,64) ->
  conv1x1(256->64) + bn + relu -> conv3x3(64->100) + bn -> pixel_shuffle(2)
  -> W (2,25,128,128) -> softmax(clip(W)^p) over 25 taps
  out[b,c,y,x] = sum_{ki,kj} W[b,(ki,kj),y,x] * Xpad[b,c,y//2+ki-2,x//2+kj-2]

Strategy (pure data-parallel over h, 8 low-res rows / core):
  * conv1x1 / conv3x3 as bf16 GEMMs (channels on partitions).
  * softmax via ACT transcendentals; tap-sums via a 100x4 selection matmul;
    reciprocal on [4,512]; denominator broadcast back to 100 partitions via
    a 4x100 selection matmul (no DRAM bounce).
  * The per-pixel 25-tap weighted sum is a banded matmul per output row h:
    out[c,(ry,x)] = sum_p XT_r[p,c] * B_ki[p,(ry,x)] accumulated over ki,
    where B_ki[w+kj, ry*128+2w+rx] = Wnorm[(ki,kj,ry,rx), h, w].  B is built
    ON-CHIP: PE-transpose Wnorm rows -> 5 partition-shift matmuls (constant
    shift matrices) -> one GPSIMD local_scatter per (b,h) placing the
    diagonal bands (per-partition indices, zeros implicit).
  * XT_r strips come from PE transposes of the input rows.

kernel(**inputs) takes the FULL inputs and returns the FULL output.
"""

import numpy as np
import ml_dtypes

SCALE = 2
K_UP = 5
B, C, H, W = 2, 256, 64, 64
N_CORES = 8
HS = H // N_CORES            # 8 low-res rows per core
XROWS = HS + 4               # 12 rows (with +-2 halo)
WP = W + 4                   # 68 (w padded by 2 each side)
CMID, CENC = 64, 100
NSLOT = K_UP * 2 * W * SCALE // 2  # bts columns per ki = 256
BTN = K_UP * 256             # 1280 elems per bts row
LCH = 80                     # local_scatter channels (68 rounded up to 16x)

_STATE = {}


def _build_nc():
    import concourse.bass as bass
    import concourse.tile as tile
    from concourse import mybir
    from concourse.vector_clock import ScopedClock
    from concourse.tile_rust import add_dep_helper

    # --- workaround: this walrus build rejects >1 sync-wait on CTRL-class
    # instructions; split the Tile tail-drain waits into 1-wait NOPs. ---
    def patched_drain_and_barrier(self, tick_clock, wait_clock):
        maxw = 1
        carrier = self.nc.sync.nop()
        wait_clock.add_sem_waits(
            carrier.ins, ScopedClock({None: tick_clock.global_clock})
        )
        si = carrier.ins.sync_info
        waits = list(si.on_wait) if si is not None else []
        if len(waits) > maxw:
            si.on_wait = waits[:maxw]
            carrier.ins.sync_info = si
            rest = waits[maxw:]
            for i in range(0, len(rest), maxw):
                n = self.nc.sync.nop()
                n.ins.sync_info = mybir.SyncInfo(
                    on_wait=rest[i : i + maxw], on_update=[]
                )
        self.nc.sync.drain()
        self.nc.all_engine_barrier()
        assert self.sems is not None
        popped = self.nc._tile_sem_poison_stack.pop()
        assert popped is self._sem_poison
        self.nc.clear_and_free_semaphores(list(self.sems.allocated().values()))
        self.nc.all_engine_barrier()

    tile.TileContext._drain_and_barrier = patched_drain_and_barrier

    # --- workaround #2: the same walrus build accepts at most ONE sync wait
    # on ANY instruction.  Post-process the serialized BIR: hoist excess
    # waits onto single-wait NoOps inserted just before, on the same engine
    # (same program point, so semantics are unchanged). ---
    import orjson

    def _split_waits_json(raw: bytes) -> bytes:
        j = orjson.loads(raw)
        n = 0
        changed = False
        for fn in j["functions"]:
            for bb in fn["blocks"]:
                out = []
                for ins in bb["instructions"]:
                    si = ins.get("sync_info")
                    waits = si.get("on_wait") if si else None
                    if waits and len(waits) > 1:
                        changed = True
                        for wt in waits[:-1]:
                            n += 1
                            out.append(
                                {
                                    "debug": ins.get("debug", 0),
                                    "engine": ins["engine"],
                                    "ins": [],
                                    "outs": [],
                                    "name": f"WSPL-{n}",
                                    "opcode": "NoOp",
                                    "sync_info": {"on_update": [], "on_wait": [wt]},
                                }
                            )
                        si["on_wait"] = [waits[-1]]
                    out.append(ins)
                bb["instructions"] = out
        return orjson.dumps(j) if changed else raw

    if not getattr(bass.Bass.to_json_bytes, "_wait_split", False):
        _orig_tjb = bass.Bass.to_json_bytes

        def patched_to_json_bytes(self):
            return _split_waits_json(_orig_tjb(self))

        patched_to_json_bytes._wait_split = True
        bass.Bass.to_json_bytes = patched_to_json_bytes

    f32 = mybir.dt.float32
    bf16 = mybir.dt.bfloat16
    i16 = mybir.dt.int16
    AF = mybir.ActivationFunctionType

    nc = bass.Bass()

    # ---- parameters ----
    xh = nc.declare_dram_parameter("xh", [B, C, XROWS, WP], bf16, isOutput=False)
    xtin = nc.declare_dram_parameter(
        "xtin", [B, XROWS, WP, 256], bf16, isOutput=False
    )
    comp_wT = nc.declare_dram_parameter(
        "comp_wT", [2, 128, CMID], bf16, isOutput=False
    )
    enc_wT = nc.declare_dram_parameter(
        "enc_wT", [9, CMID, CENC], bf16, isOutput=False
    )
    c_g = nc.declare_dram_parameter("comp_gamma", [CMID], f32, isOutput=False)
    c_b = nc.declare_dram_parameter("comp_beta", [CMID], f32, isOutput=False)
    c_m = nc.declare_dram_parameter("comp_mean", [CMID], f32, isOutput=False)
    c_v = nc.declare_dram_parameter("comp_var", [CMID], f32, isOutput=False)
    e_g = nc.declare_dram_parameter("enc_gamma", [CENC], f32, isOutput=False)
    e_b = nc.declare_dram_parameter("enc_beta", [CENC], f32, isOutput=False)
    e_m = nc.declare_dram_parameter("enc_mean", [CENC], f32, isOutput=False)
    e_v = nc.declare_dram_parameter("enc_var", [CENC], f32, isOutput=False)
    p_in = nc.declare_dram_parameter("power_p", [1], f32, isOutput=False)
    sel = nc.declare_dram_parameter("sel", [CENC, 4], f32, isOutput=False)
    selT = nc.declare_dram_parameter("selT", [4, CENC], f32, isOutput=False)
    ident = nc.declare_dram_parameter("ident", [128, 128], bf16, isOutput=False)
    y1mask = nc.declare_dram_parameter("y1mask", [660], bf16, isOutput=False)
    shmat = nc.declare_dram_parameter("shmat", [K_UP, W, LCH], bf16, isOutput=False)
    lsidx = nc.declare_dram_parameter("lsidx", [LCH, CENC], i16, isOutput=False)

    out = nc.declare_dram_parameter(
        "out", [B, C, 2 * HS, 2 * W], f32, isOutput=True
    )
    # donated-zero scratch output (never read host-side)
    pscr = nc.declare_dram_parameter("pscr", [1], f32, isOutput=True)

    def dram_ap(param, offset, dims):
        return bass.AP(tensor=param, offset=offset, ap=[list(d) for d in dims])

    with tile.TileContext(nc) as tc:
        import contextlib

        ctx = contextlib.ExitStack()
        const = ctx.enter_context(tc.tile_pool(name="const", bufs=1))
        sm = ctx.enter_context(tc.tile_pool(name="sm", bufs=2))
        dp = ctx.enter_context(tc.tile_pool(name="dp", bufs=4))
        btp = ctx.enter_context(tc.tile_pool(name="btp", bufs=10))
        op = ctx.enter_context(tc.tile_pool(name="op", bufs=4))
        ps_big = ctx.enter_context(tc.tile_pool(name="ps_big", bufs=2, space="PSUM"))
        ps_bf = ctx.enter_context(tc.tile_pool(name="ps_bf", bufs=2, space="PSUM"))
        ps_sh = ctx.enter_context(tc.tile_pool(name="ps_sh", bufs=2, space="PSUM"))
        ps_e = ctx.enter_context(tc.tile_pool(name="ps_e", bufs=2, space="PSUM"))

        # ---- constants in SBUF ----
        ident_sb = const.tile([128, 128], bf16, tag="ident")
        nc.sync.dma_start(out=ident_sb[:, :], in_=ident[:, :])
        sel_sb = const.tile([CENC, 4], f32, tag="sel")
        nc.sync.dma_start(out=sel_sb[:, :], in_=sel[:, :])
        selT_sb = const.tile([4, CENC], f32, tag="selT")
        nc.sync.dma_start(out=selT_sb[:, :], in_=selT[:, :])
        mask_sb = const.tile([CMID, 10, 66], bf16, tag="mask")
        nc.sync.dma_start(
            out=mask_sb[:, :, :],
            in_=dram_ap(y1mask, 0, [[0, CMID], [66, 10], [1, 66]]),
        )
        sh_all = const.tile([W, K_UP, LCH], bf16, tag="shmat")
        nc.sync.dma_start(
            out=sh_all[:, :, :],
            in_=dram_ap(shmat, 0, [[LCH, W], [W * LCH, K_UP], [1, LCH]]),
        )
        lsidx_sb = const.tile([LCH, CENC], i16, tag="lsidx")
        nc.sync.dma_start(out=lsidx_sb[:, :], in_=lsidx[:, :])

        # conv1x1 weights (host-transposed): lhsT [cin(128) x cout(64)]
        comp_bf = []
        for ct in range(2):
            cb = const.tile([128, CMID], bf16, tag=f"comp_bf{ct}")
            nc.sync.dma_start(
                out=cb[:, :],
                in_=dram_ap(comp_wT, ct * 128 * CMID, [[CMID, 128], [1, CMID]]),
            )
            comp_bf.append(cb)

        # conv3x3 weights (host-transposed): lhsT [cin(64) x cout(100)]
        enc_bf = []
        for j in range(9):
            eb = const.tile([CMID, CENC], bf16, tag=f"enc_bf{j}")
            nc.sync.dma_start(
                out=eb[:, :],
                in_=dram_ap(
                    enc_wT, j * CMID * CENC, [[CENC, CMID], [1, CENC]]
                ),
            )
            enc_bf.append(eb)

        # ---- batchnorm fold: inv = gamma/sqrt(var+eps), shift = beta-mean*inv
        def bn_fold(gamma, beta, mean, var, n, tagp):
            g = const.tile([n, 1], f32, tag=f"{tagp}g")
            bt = const.tile([n, 1], f32, tag=f"{tagp}b")
            m = const.tile([n, 1], f32, tag=f"{tagp}m")
            v = const.tile([n, 1], f32, tag=f"{tagp}v")
            for t, src in ((g, gamma), (bt, beta), (m, mean), (v, var)):
                nc.sync.dma_start(out=t[:, :], in_=dram_ap(src, 0, [[1, n]]))
            eps = const.tile([n, 1], f32, tag=f"{tagp}e")
            nc.vector.memset(eps[:, :], 1e-5)
            std = const.tile([n, 1], f32, tag=f"{tagp}s")
            nc.scalar.activation(std[:, :], v[:, :], AF.Sqrt, bias=eps[:, :])
            rstd = const.tile([n, 1], f32, tag=f"{tagp}r")
            nc.vector.reciprocal(rstd[:, :], std[:, :])
            inv = const.tile([n, 1], f32, tag=f"{tagp}i")
            nc.vector.tensor_mul(inv[:, :], g[:, :], rstd[:, :])
            tmp = const.tile([n, 1], f32, tag=f"{tagp}t")
            nc.vector.tensor_mul(tmp[:, :], m[:, :], inv[:, :])
            shift = const.tile([n, 1], f32, tag=f"{tagp}h")
            nc.vector.tensor_sub(shift[:, :], bt[:, :], tmp[:, :])
            return inv, shift

        inv1, shift1 = bn_fold(c_g, c_b, c_m, c_v, CMID, "bn1")
        inv2, shift2 = bn_fold(e_g, e_b, e_m, e_v, CENC, "bn2")

        # ---- p = clip(power_p, 1e-5), broadcast to [100,1] via DRAM bounce
        p_sb = const.tile([1, 1], f32, tag="p")
        nc.sync.dma_start(out=p_sb[:, :], in_=dram_ap(p_in, 0, [[1, 1]]))
        nc.vector.tensor_scalar_max(p_sb[:, :], p_sb[:, :], 1e-5)
        p_wr = nc.sync.dma_start(out=dram_ap(pscr, 0, [[1, 1]]), in_=p_sb[:, :])
        pb_sb = const.tile([CENC, 1], f32, tag="pb")
        p_rd = nc.sync.dma_start(
            out=pb_sb[:, :], in_=dram_ap(pscr, 0, [[0, CENC], [1, 1]])
        )
        add_dep_helper(p_rd.ins, p_wr.ins, sync=True, reason="pscr RAW")

        # ---- X rows in SBUF (bf16, already padded on host) ----
        xbf = [[None, None] for _ in range(B)]
        for b in range(B):
            for ct in range(2):
                t = const.tile([128, XROWS, WP], bf16, tag=f"xbf{b}{ct}")
                nc.sync.dma_start(
                    out=t[:, :, :],
                    in_=dram_ap(
                        xh,
                        b * C * XROWS * WP + ct * 128 * XROWS * WP,
                        [[XROWS * WP, 128], [1, XROWS * WP]],
                    ),
                )
                xbf[b][ct] = t

        # ---- Y1 tiles (zeroed once; borders stay zero) ----
        y1 = []
        for b in range(B):
            t = const.tile([CMID, 10, 66], bf16, tag=f"y1_{b}")
            nc.vector.memset(t[:, :, :], 0.0)
            y1.append(t)

        # ---- X^T strips loaded pre-transposed from host ----
        xts_all = []
        for b in range(B):
            t = const.tile([WP, XROWS, 256], bf16, tag=f"xts{b}")
            nc.sync.dma_start(
                out=t[:, :, :],
                in_=dram_ap(
                    xtin,
                    b * XROWS * WP * 256,
                    [[256, WP], [WP * 256, XROWS], [1, 256]],
                ),
            )
            xts_all.append(t)

        for b in range(B):
            # ===== conv1x1 + bn1 + relu =====
            for half in range(2):
                pcb = ps_big.tile([CENC, HS * W], f32, tag="big")
                pc = pcb[0:CMID, 0:320]
                for ct in range(2):
                    nc.tensor.matmul(
                        pc,
                        comp_bf[ct][:, :],
                        xbf[b][ct][:, 1 + 5 * half : 6 + 5 * half, 2 : 2 + W],
                        start=(ct == 0),
                        stop=(ct == 1),
                    )
                nc.scalar.activation(
                    y1[b][:, 5 * half : 5 * half + 5, 1 : 1 + W],
                    pc,
                    AF.Relu,
                    bias=shift1[:, :],
                    scale=inv1[:, :],
                )
            # zero out-of-image rows / padding cols
            nc.vector.tensor_mul(y1[b][:, :, :], y1[b][:, :, :], mask_sb[:, :, :])

            # ===== conv3x3 + bn2 =====
            pc3 = ps_big.tile([CENC, HS * W], f32, tag="big")
            jj = 0
            for dy in (-1, 0, 1):
                for dx in (-1, 0, 1):
                    nc.tensor.matmul(
                        pc3[:, :],
                        enc_bf[jj][:, :],
                        y1[b][:, 1 + dy : 9 + dy, 1 + dx : 1 + dx + W],
                        start=(jj == 0),
                        stop=(jj == 8),
                    )
                    jj += 1
            w_sb = sm.tile([CENC, HS * W], f32, tag="w")
            nc.scalar.activation(
                w_sb[:, :], pc3[:, :], AF.Identity, bias=shift2[:, :], scale=inv2[:, :]
            )

            # ===== power + softmax numerator =====
            nc.vector.tensor_scalar_max(w_sb[:, :], w_sb[:, :], 1e-5)
            nc.scalar.activation(w_sb[:, :], w_sb[:, :], AF.Ln)
            nc.scalar.activation(w_sb[:, :], w_sb[:, :], AF.Exp, scale=pb_sb[:, :])
            e_sb = sm.tile([CENC, HS * W], f32, tag="e")
            nc.scalar.activation(e_sb[:, :], w_sb[:, :], AF.Exp)

            # ===== tap-sums, reciprocal, broadcast, normalize =====
            psb = ps_big.tile([CENC, HS * W], f32, tag="big")
            ps = psb[0:4, :]
            nc.tensor.matmul(ps, sel_sb[:, :], e_sb[:, :], start=True, stop=True)
            r4_sb = sm.tile([4, HS * W], f32, tag="r4")
            nc.vector.reciprocal_approx_fast(r4_sb[:, :], ps)
            rb_ps = ps_big.tile([CENC, HS * W], f32, tag="big")
            nc.tensor.matmul(
                rb_ps[:, :], selT_sb[:, :], r4_sb[:, :], start=True, stop=True
            )
            en_sb = const.tile([CENC, HS, W], bf16, tag=f"en{b}")
            nc.vector.tensor_mul(
                en_sb[:, :, :],
                e_sb[:, :].rearrange("p (a b) -> p a b", b=W),
                rb_ps[:, :].rearrange("p (a b) -> p a b", b=W),
            )

            # ===== banded-matrix build: transpose + shifts + local_scatter ===
            bts_all = []
            tp_sbs = {}

            def emit_transpose(h):
                tpt = ps_bf.tile([W, CENC], bf16, tag="bf")
                tp_ps = tpt[:, :]
                nc.tensor.transpose(
                    tp_ps, en_sb[:, h, :], ident_sb[:CENC, :CENC]
                )
                t = dp.tile([W, CENC], bf16, tag="tps")
                nc.vector.tensor_copy(t[:, :], tp_ps)
                tp_sbs[h] = t

            emit_transpose(0)
            for h in range(HS):
                if h + 1 < HS:
                    emit_transpose(h + 1)
                tp_sb = tp_sbs.pop(h)
                sh_ps = ps_sh.tile([LCH, CENC], f32, tag="sh")
                tp_v = tp_sb[:, :].rearrange("p (a b) -> p a b", b=20)
                for s in range(K_UP):
                    nc.tensor.matmul(
                        sh_ps[:, 20 * s : 20 * (s + 1)],
                        sh_all[:, s, :],
                        tp_v[:, :, 4 * s : 4 * (s + 1)],
                        start=True,
                        stop=True,
                    )
                data_sb = dp.tile([LCH, CENC], bf16, tag="data")
                nc.vector.tensor_copy(data_sb[:, :], sh_ps[:, :])
                bts = btp.tile([LCH, BTN], bf16, tag="bts")
                nc.gpsimd.local_scatter(
                    out_ap=bts[:, :],
                    data_ap=data_sb[:, :],
                    idxs_ap=lsidx_sb[:, :],
                    channels=LCH,
                    num_elems=BTN,
                    num_idxs=CENC,
                )
                bts_all.append(bts)

            # ===== banded einsum =====
            for h in range(HS):
                bts = bts_all[h]
                for ct in range(2):
                    pe = ps_e.tile([128, 256], f32, tag="pe")
                    for ki in range(K_UP):
                        nc.tensor.matmul(
                            pe[:, :],
                            xts_all[b][:, h + ki, ct * 128 : (ct + 1) * 128],
                            bts[0:WP, ki * 256 : (ki + 1) * 256],
                            start=(ki == 0),
                            stop=(ki == K_UP - 1),
                        )
                    o_sb = op.tile([128, 256], f32, tag="osb")
                    if ct == 0:
                        nc.scalar.activation(o_sb[:, :], pe[:, :], AF.Identity)
                    else:
                        nc.vector.tensor_copy(o_sb[:, :], pe[:, :])
                    nc.sync.dma_start(
                        out=dram_ap(
                            out,
                            b * C * 2 * HS * 2 * W
                            + ct * 128 * 2 * HS * 2 * W
                            + 2 * h * 2 * W,
                            [[2 * HS * 2 * W, 128], [2 * W, 2], [1, 2 * W]],
                        ),
                        in_=o_sb[:, :],
                    )

        ctx.close()

    # ---- Bacc-style finishing passes: library loads + ISA assembly ----
    from concourse.library_config import all_libraries, standard
    import bass_rust as _bass_rust

    lib_mask = {}
    for lib in all_libraries:
        for it in lib.instructions:
            lib_mask[it] = lib_mask.get(it, 0) | (1 << lib.index)
    _bass_rust.insert_library_loads(nc, lib_mask, len(all_libraries), standard.index)
    mybir.codegen_inst_isa_subclasses(nc)

    return nc


def _get_nc():
    if "nc" not in _STATE:
        _STATE["nc"] = _build_nc()
    return _STATE["nc"]


def _make_in_maps(inputs):
    X = np.asarray(inputs["X"], dtype=np.float32)
    Xp = np.pad(X, ((0, 0), (0, 0), (2, 2), (2, 2)))
    sel = np.zeros((CENC, 4), np.float32)
    for p in range(CENC):
        sel[p, p % 4] = 1.0
    ident = np.eye(128, dtype=ml_dtypes.bfloat16)
    shmat = np.zeros((K_UP, W, LCH), np.float32)
    for s in range(K_UP):
        for w in range(W):
            shmat[s, w, w + s] = 1.0
    lsidx = np.full((LCH, CENC), -1, np.int16)
    for p in range(WP):
        for s in range(K_UP):
            w = p - s
            if 0 <= w < W:
                for ki in range(K_UP):
                    for u in range(4):
                        ry, rx = u // 2, u % 2
                        c = s * 20 + ki * 4 + u
                        lsidx[p, c] = ki * 256 + ry * 128 + 2 * w + rx
    comp_wT = (
        np.asarray(inputs["comp_w"], np.float32)[:, :, 0, 0]
        .T.reshape(2, 128, CMID)
        .astype(ml_dtypes.bfloat16)
    )
    enc_wT = (
        np.asarray(inputs["enc_w"], np.float32)
        .reshape(CENC, CMID, 9)
        .transpose(2, 1, 0)
        .copy()
        .astype(ml_dtypes.bfloat16)
    )
    common = {
        "comp_wT": comp_wT,
        "enc_wT": enc_wT,
        "comp_gamma": np.asarray(inputs["comp_gamma"], np.float32),
        "comp_beta": np.asarray(inputs["comp_beta"], np.float32),
        "comp_mean": np.asarray(inputs["comp_mean"], np.float32),
        "comp_var": np.asarray(inputs["comp_var"], np.float32),
        "enc_gamma": np.asarray(inputs["enc_gamma"], np.float32),
        "enc_beta": np.asarray(inputs["enc_beta"], np.float32),
        "enc_mean": np.asarray(inputs["enc_mean"], np.float32),
        "enc_var": np.asarray(inputs["enc_var"], np.float32),
        "power_p": np.asarray(inputs["power_p"], np.float32),
        "sel": sel,
        "selT": sel.T.copy(),
        "ident": ident,
        "shmat": shmat.astype(ml_dtypes.bfloat16),
        "lsidx": lsidx,
    }
    in_maps = []
    for core in range(N_CORES):
        r0 = HS * core
        xh = np.ascontiguousarray(Xp[:, :, r0 : r0 + XROWS, :]).astype(
            ml_dtypes.bfloat16
        )
        mask = np.zeros((10, 66), np.float32)
        for rr in range(10):
            grow = r0 - 1 + rr
            if 0 <= grow < H:
                mask[rr, 1 : 1 + W] = 1.0
        m = dict(common)
        m["xh"] = xh
        m["xtin"] = np.ascontiguousarray(xh.transpose(0, 2, 3, 1))
        m["y1mask"] = mask.reshape(660).astype(ml_dtypes.bfloat16)
        in_maps.append(m)
    return in_maps


def _run(inputs, trace=False):
    from concourse.bass_utils import run_bass_kernel_spmd

    if trace:
        import sys, os
        sys.path.insert(0, os.path.dirname(os.path.abspath(__file__)))
        import hookshim  # noqa: F401

    nc = _get_nc()
    in_maps = _make_in_maps(inputs)
    res = run_bass_kernel_spmd(
        nc, in_maps, core_ids=list(range(N_CORES)), trace=trace
    )
    out = np.concatenate([res.results[c]["out"] for c in range(N_CORES)], axis=2)
    return out, res


def kernel(**inputs):
    out, _ = _run(inputs, trace=False)
    return out


# revision 36
# speedup vs baseline: 18.7115x; 1.1215x over previous
"""CARAFE (content-aware upsample, power-normalized softmax) on 8 TRN2 cores.

Math (reference.py): X (2,256,64,64) ->
  conv1x1(256->64) + bn + relu -> conv3x3(64->100) + bn -> pixel_shuffle(2)
  -> W (2,25,128,128) -> softmax(clip(W)^p) over 25 taps
  out[b,c,y,x] = sum_{ki,kj} W[b,(ki,kj),y,x] * Xpad[b,c,y//2+ki-2,x//2+kj-2]

Strategy (pure data-parallel over h, 8 low-res rows / core):
  * conv1x1 / conv3x3 as bf16 GEMMs (channels on partitions).
  * softmax via ACT transcendentals; tap-sums via a 100x4 selection matmul;
    reciprocal on [4,512]; denominator broadcast back to 100 partitions via
    a 4x100 selection matmul (no DRAM bounce).
  * The per-pixel 25-tap weighted sum is a banded matmul per output row h:
    out[c,(ry,x)] = sum_p XT_r[p,c] * B_ki[p,(ry,x)] accumulated over ki,
    where B_ki[w+kj, ry*128+2w+rx] = Wnorm[(ki,kj,ry,rx), h, w].  B is built
    ON-CHIP: PE-transpose Wnorm rows -> 5 partition-shift matmuls (constant
    shift matrices) -> one GPSIMD local_scatter per (b,h) placing the
    diagonal bands (per-partition indices, zeros implicit).
  * XT_r strips come from PE transposes of the input rows.

kernel(**inputs) takes the FULL inputs and returns the FULL output.
"""

import numpy as np
import ml_dtypes

SCALE = 2
K_UP = 5
B, C, H, W = 2, 256, 64, 64
N_CORES = 8
HS = H // N_CORES            # 8 low-res rows per core
XROWS = HS + 4               # 12 rows (with +-2 halo)
WP = W + 4                   # 68 (w padded by 2 each side)
CMID, CENC = 64, 100
NSLOT = K_UP * 2 * W * SCALE // 2  # bts columns per ki = 256
BTN = K_UP * 256             # 1280 elems per bts row
LCH = 80                     # local_scatter channels (68 rounded up to 16x)

_STATE = {}


def _build_nc():
    import concourse.bass as bass
    import concourse.tile as tile
    from concourse import mybir
    from concourse.vector_clock import ScopedClock
    from concourse.tile_rust import add_dep_helper

    # --- workaround: this walrus build rejects >1 sync-wait on CTRL-class
    # instructions; split the Tile tail-drain waits into 1-wait NOPs. ---
    def patched_drain_and_barrier(self, tick_clock, wait_clock):
        maxw = 1
        carrier = self.nc.sync.nop()
        wait_clock.add_sem_waits(
            carrier.ins, ScopedClock({None: tick_clock.global_clock})
        )
        si = carrier.ins.sync_info
        waits = list(si.on_wait) if si is not None else []
        if len(waits) > maxw:
            si.on_wait = waits[:maxw]
            carrier.ins.sync_info = si
            rest = waits[maxw:]
            for i in range(0, len(rest), maxw):
                n = self.nc.sync.nop()
                n.ins.sync_info = mybir.SyncInfo(
                    on_wait=rest[i : i + maxw], on_update=[]
                )
        self.nc.sync.drain()
        self.nc.all_engine_barrier()
        assert self.sems is not None
        popped = self.nc._tile_sem_poison_stack.pop()
        assert popped is self._sem_poison
        self.nc.clear_and_free_semaphores(list(self.sems.allocated().values()))
        self.nc.all_engine_barrier()

    tile.TileContext._drain_and_barrier = patched_drain_and_barrier

    # --- workaround #2: the same walrus build accepts at most ONE sync wait
    # on ANY instruction.  Post-process the serialized BIR: hoist excess
    # waits onto single-wait NoOps inserted just before, on the same engine
    # (same program point, so semantics are unchanged). ---
    import orjson

    def _split_waits_json(raw: bytes) -> bytes:
        j = orjson.loads(raw)
        n = 0
        changed = False
        for fn in j["functions"]:
            for bb in fn["blocks"]:
                out = []
                for ins in bb["instructions"]:
                    si = ins.get("sync_info")
                    waits = si.get("on_wait") if si else None
                    if waits and len(waits) > 1:
                        changed = True
                        for wt in waits[:-1]:
                            n += 1
                            out.append(
                                {
                                    "debug": ins.get("debug", 0),
                                    "engine": ins["engine"],
                                    "ins": [],
                                    "outs": [],
                                    "name": f"WSPL-{n}",
                                    "opcode": "NoOp",
                                    "sync_info": {"on_update": [], "on_wait": [wt]},
                                }
                            )
                        si["on_wait"] = [waits[-1]]
                    out.append(ins)
                bb["instructions"] = out
        return orjson.dumps(j) if changed else raw

    if not getattr(bass.Bass.to_json_bytes, "_wait_split", False):
        _orig_tjb = bass.Bass.to_json_bytes

        def patched_to_json_bytes(self):
            return _split_waits_json(_orig_tjb(self))

        patched_to_json_bytes._wait_split = True
        bass.Bass.to_json_bytes = patched_to_json_bytes

    f32 = mybir.dt.float32
    bf16 = mybir.dt.bfloat16
    i16 = mybir.dt.int16
    AF = mybir.ActivationFunctionType

    nc = bass.Bass()

    # ---- parameters ----
    # Packed inputs (host-prepared layouts; see _make_in_maps):
    #  xh    [B, 2, 128, XROWS*WP]   bf16  image, channels on partitions
    #  xtin  [B, XROWS, WP, 256]     bf16  image pre-transposed (w on partitions)
    #  bfpk  [128, BFPK]             bf16  ident|mask|shmat|encT|compT
    #  f32pk [CENC, F32PK]           f32   sel|selT|bn params
    #  lsidx [LCH, CENC]             i16   local_scatter indices
    BFPK = 128 + 660 + K_UP * LCH + 9 * CENC + 2 * CMID
    F32PK = 4 + CENC + 4 + 4
    xh = nc.declare_dram_parameter("xh", [B, 2, 128, XROWS * WP], bf16, isOutput=False)
    xtin = nc.declare_dram_parameter(
        "xtin", [B, XROWS, WP, 256], bf16, isOutput=False
    )
    bfpk = nc.declare_dram_parameter("bfpk", [128, BFPK], bf16, isOutput=False)
    f32pk = nc.declare_dram_parameter("f32pk", [CENC, F32PK], f32, isOutput=False)
    p_in = nc.declare_dram_parameter("power_p", [1], f32, isOutput=False)
    lsidx = nc.declare_dram_parameter("lsidx", [LCH, CENC], i16, isOutput=False)

    out = nc.declare_dram_parameter(
        "out", [B, C, 2 * HS, 2 * W], f32, isOutput=True
    )
    # donated-zero scratch output (never read host-side)
    pscr = nc.declare_dram_parameter("pscr", [1], f32, isOutput=True)

    def dram_ap(param, offset, dims):
        return bass.AP(tensor=param, offset=offset, ap=[list(d) for d in dims])

    with tile.TileContext(nc) as tc:
        import contextlib

        ctx = contextlib.ExitStack()
        const = ctx.enter_context(tc.tile_pool(name="const", bufs=1))
        sm = ctx.enter_context(tc.tile_pool(name="sm", bufs=2))
        dp = ctx.enter_context(tc.tile_pool(name="dp", bufs=4))
        btp = ctx.enter_context(tc.tile_pool(name="btp", bufs=10))
        op = ctx.enter_context(tc.tile_pool(name="op", bufs=4))
        ps_big = ctx.enter_context(tc.tile_pool(name="ps_big", bufs=2, space="PSUM"))
        ps_bf = ctx.enter_context(tc.tile_pool(name="ps_bf", bufs=2, space="PSUM"))
        ps_sh = ctx.enter_context(tc.tile_pool(name="ps_sh", bufs=2, space="PSUM"))
        ps_e = ctx.enter_context(tc.tile_pool(name="ps_e", bufs=2, space="PSUM"))

        # ---- packed constants in SBUF (one DMA per dtype class) ----
        bf_sb = const.tile([128, BFPK], bf16, tag="bfpk")
        nc.sync.dma_start(out=bf_sb[:, :], in_=bfpk[:, :])
        f32_sb = const.tile([CENC, F32PK], f32, tag="f32pk")
        nc.scalar.dma_start(out=f32_sb[:, :], in_=f32pk[:, :])
        lsidx_sb = const.tile([LCH, CENC], i16, tag="lsidx")
        nc.scalar.dma_start(out=lsidx_sb[:, :], in_=lsidx[:, :])

        o_id = 0
        ident_sb = bf_sb[:, 0:128]
        o_id += 128
        mask_v = bf_sb[0:CMID, o_id : o_id + 660].rearrange(
            "p (a b) -> p a b", b=66
        )
        o_id += 660
        sh_all = bf_sb[0:W, o_id : o_id + K_UP * LCH].rearrange(
            "p (a b) -> p a b", b=LCH
        )
        o_id += K_UP * LCH
        enc_bf = []
        for j in range(9):
            enc_bf.append(bf_sb[0:CMID, o_id : o_id + CENC])
            o_id += CENC
        comp_bf = []
        for ct in range(2):
            comp_bf.append(bf_sb[:, o_id : o_id + CMID])
            o_id += CMID

        sel_sb = f32_sb[:, 0:4]
        selT_sb = f32_sb[0:4, 4 : 4 + CENC]

        # ---- batchnorm fold: inv = gamma/sqrt(var+eps), shift = beta-mean*inv
        def bn_fold(n, col0, tagp):
            g = f32_sb[0:n, col0 : col0 + 1]
            bt = f32_sb[0:n, col0 + 1 : col0 + 2]
            m = f32_sb[0:n, col0 + 2 : col0 + 3]
            v = f32_sb[0:n, col0 + 3 : col0 + 4]
            eps = const.tile([n, 1], f32, tag=f"{tagp}e")
            nc.vector.memset(eps[:, :], 1e-5)
            std = const.tile([n, 1], f32, tag=f"{tagp}s")
            nc.scalar.activation(std[:, :], v, AF.Sqrt, bias=eps[:, :])
            rstd = const.tile([n, 1], f32, tag=f"{tagp}r")
            nc.vector.reciprocal(rstd[:, :], std[:, :])
            inv = const.tile([n, 1], f32, tag=f"{tagp}i")
            nc.vector.tensor_mul(inv[:, :], g, rstd[:, :])
            tmp = const.tile([n, 1], f32, tag=f"{tagp}t")
            nc.vector.tensor_mul(tmp[:, :], m, inv[:, :])
            shift = const.tile([n, 1], f32, tag=f"{tagp}h")
            nc.vector.tensor_sub(shift[:, :], bt, tmp[:, :])
            return inv, shift

        inv1, shift1 = bn_fold(CMID, 4 + CENC, "bn1")
        inv2, shift2 = bn_fold(CENC, 4 + CENC + 4, "bn2")

        # ---- p = clip(power_p, 1e-5), broadcast to [100,1] via DRAM bounce
        p_sb = const.tile([1, 1], f32, tag="p")
        nc.sync.dma_start(out=p_sb[:, :], in_=dram_ap(p_in, 0, [[1, 1]]))
        nc.vector.tensor_scalar_max(p_sb[:, :], p_sb[:, :], 1e-5)
        p_wr = nc.sync.dma_start(out=dram_ap(pscr, 0, [[1, 1]]), in_=p_sb[:, :])
        pb_sb = const.tile([CENC, 1], f32, tag="pb")
        p_rd = nc.sync.dma_start(
            out=pb_sb[:, :], in_=dram_ap(pscr, 0, [[0, CENC], [1, 1]])
        )
        add_dep_helper(p_rd.ins, p_wr.ins, sync=True, reason="pscr RAW")

        # ---- X rows in SBUF (bf16, already padded on host) ----
        xbf = []
        for b in range(B):
            t = const.tile([128, 2, XROWS, WP], bf16, tag=f"xbf{b}")
            eng = nc.sync if b == 0 else nc.scalar
            eng.dma_start(
                out=t[:, :, :, :],
                in_=dram_ap(
                    xh,
                    b * 2 * 128 * XROWS * WP,
                    [
                        [XROWS * WP, 128],
                        [128 * XROWS * WP, 2],
                        [1, XROWS * WP],
                    ],
                ),
            )
            xbf.append(t)

        # ---- Y1 tiles (zeroed once; borders stay zero) ----
        y1 = []
        for b in range(B):
            t = const.tile([CMID, 10, 66], bf16, tag=f"y1_{b}")
            nc.vector.memset(t[:, :, :], 0.0)
            y1.append(t)

        # ---- X^T strips loaded pre-transposed from host ----
        xts_all = []
        for b in range(B):
            t = const.tile([WP, XROWS, 256], bf16, tag=f"xts{b}")
            eng = nc.scalar if b == 0 else nc.sync
            eng.dma_start(
                out=t[:, :, :],
                in_=dram_ap(
                    xtin,
                    b * XROWS * WP * 256,
                    [[256, WP], [WP * 256, XROWS], [1, 256]],
                ),
            )
            xts_all.append(t)

        for b in range(B):
            # ===== conv1x1 + bn1 + relu =====
            for half in range(2):
                pcb = ps_big.tile([CENC, HS * W], f32, tag="big")
                pc = pcb[0:CMID, 0:320]
                for ct in range(2):
                    nc.tensor.matmul(
                        pc,
                        comp_bf[ct],
                        xbf[b][:, ct, 1 + 5 * half : 6 + 5 * half, 2 : 2 + W],
                        start=(ct == 0),
                        stop=(ct == 1),
                    )
                nc.scalar.activation(
                    y1[b][:, 5 * half : 5 * half + 5, 1 : 1 + W],
                    pc,
                    AF.Relu,
                    bias=shift1[:, :],
                    scale=inv1[:, :],
                )
            # zero out-of-image rows / padding cols
            nc.vector.tensor_mul(y1[b][:, :, :], y1[b][:, :, :], mask_v)

            # ===== conv3x3 + bn2 =====
            pc3 = ps_big.tile([CENC, HS * W], f32, tag="big")
            jj = 0
            for dy in (-1, 0, 1):
                for dx in (-1, 0, 1):
                    nc.tensor.matmul(
                        pc3[:, :],
                        enc_bf[jj],
                        y1[b][:, 1 + dy : 9 + dy, 1 + dx : 1 + dx + W],
                        start=(jj == 0),
                        stop=(jj == 8),
                    )
                    jj += 1
            w_sb = sm.tile([CENC, HS * W], f32, tag="w")
            nc.scalar.activation(
                w_sb[:, :], pc3[:, :], AF.Identity, bias=shift2[:, :], scale=inv2[:, :]
            )

            # ===== power + softmax numerator =====
            nc.vector.tensor_scalar_max(w_sb[:, :], w_sb[:, :], 1e-5)
            nc.scalar.activation(w_sb[:, :], w_sb[:, :], AF.Ln)
            nc.scalar.activation(w_sb[:, :], w_sb[:, :], AF.Exp, scale=pb_sb[:, :])
            e_sb = sm.tile([CENC, HS * W], f32, tag="e")
            nc.scalar.activation(e_sb[:, :], w_sb[:, :], AF.Exp)

            # ===== tap-sums, reciprocal, broadcast, normalize =====
            psb = ps_big.tile([CENC, HS * W], f32, tag="big")
            ps = psb[0:4, :]
            nc.tensor.matmul(ps, sel_sb, e_sb[:, :], start=True, stop=True)
            r4_sb = sm.tile([4, HS * W], f32, tag="r4")
            nc.vector.reciprocal_approx_fast(r4_sb[:, :], ps)
            rb_ps = ps_big.tile([CENC, HS * W], f32, tag="big")
            nc.tensor.matmul(
                rb_ps[:, :], selT_sb, r4_sb[:, :], start=True, stop=True
            )
            en_sb = const.tile([CENC, HS, W], bf16, tag=f"en{b}")
            nc.vector.tensor_mul(
                en_sb[:, :, :],
                e_sb[:, :].rearrange("p (a b) -> p a b", b=W),
                rb_ps[:, :].rearrange("p (a b) -> p a b", b=W),
            )

            # ===== banded-matrix build: transpose + shifts + local_scatter ===
            bts_all = []
            tp_sbs = {}

            def emit_transpose(h):
                tpt = ps_bf.tile([W, CENC], bf16, tag="bf")
                tp_ps = tpt[:, :]
                nc.tensor.transpose(
                    tp_ps, en_sb[:, h, :], ident_sb[0:CENC, 0:CENC]
                )
                t = dp.tile([W, CENC], bf16, tag="tps")
                nc.vector.tensor_copy(t[:, :], tp_ps)
                tp_sbs[h] = t

            emit_transpose(0)
            for h in range(HS):
                if h + 1 < HS:
                    emit_transpose(h + 1)
                tp_sb = tp_sbs.pop(h)
                sh_ps = ps_sh.tile([LCH, CENC], f32, tag="sh")
                tp_v = tp_sb[:, :].rearrange("p (a b) -> p a b", b=20)
                for s in range(K_UP):
                    nc.tensor.matmul(
                        sh_ps[:, 20 * s : 20 * (s + 1)],
                        sh_all[:, s, :],
                        tp_v[:, :, 4 * s : 4 * (s + 1)],
                        start=True,
                        stop=True,
                    )
                data_sb = dp.tile([LCH, CENC], bf16, tag="data")
                nc.vector.tensor_copy(data_sb[:, :], sh_ps[:, :])
                bts = btp.tile([LCH, BTN], bf16, tag="bts")
                nc.gpsimd.local_scatter(
                    out_ap=bts[:, :],
                    data_ap=data_sb[:, :],
                    idxs_ap=lsidx_sb[:, :],
                    channels=LCH,
                    num_elems=BTN,
                    num_idxs=CENC,
                )
                bts_all.append(bts)

            # ===== banded einsum =====
            for h in range(HS):
                bts = bts_all[h]
                for ct in range(2):
                    pe = ps_e.tile([128, 256], f32, tag="pe")
                    for ki in range(K_UP):
                        nc.tensor.matmul(
                            pe[:, :],
                            xts_all[b][:, h + ki, ct * 128 : (ct + 1) * 128],
                            bts[0:WP, ki * 256 : (ki + 1) * 256],
                            start=(ki == 0),
                            stop=(ki == K_UP - 1),
                        )
                    o_sb = op.tile([128, 256], f32, tag="osb")
                    if ct == 0:
                        nc.scalar.activation(o_sb[:, :], pe[:, :], AF.Identity)
                    else:
                        nc.vector.tensor_copy(o_sb[:, :], pe[:, :])
                    oeng = nc.sync if ct == 0 else nc.scalar
                    oeng.dma_start(
                        out=dram_ap(
                            out,
                            b * C * 2 * HS * 2 * W
                            + ct * 128 * 2 * HS * 2 * W
                            + 2 * h * 2 * W,
                            [[2 * HS * 2 * W, 128], [2 * W, 2], [1, 2 * W]],
                        ),
                        in_=o_sb[:, :],
                    )

        ctx.close()

    # ---- Bacc-style finishing passes: library loads + ISA assembly ----
    from concourse.library_config import all_libraries, standard
    import bass_rust as _bass_rust

    lib_mask = {}
    for lib in all_libraries:
        for it in lib.instructions:
            lib_mask[it] = lib_mask.get(it, 0) | (1 << lib.index)
    _bass_rust.insert_library_loads(nc, lib_mask, len(all_libraries), standard.index)
    mybir.codegen_inst_isa_subclasses(nc)

    return nc


def _get_nc():
    if "nc" not in _STATE:
        _STATE["nc"] = _build_nc()
    return _STATE["nc"]


def _make_in_maps(inputs):
    bf16 = ml_dtypes.bfloat16
    BFPK = 128 + 660 + K_UP * LCH + 9 * CENC + 2 * CMID
    F32PK = 4 + CENC + 4 + 4
    X = np.asarray(inputs["X"], dtype=np.float32)
    Xp = np.pad(X, ((0, 0), (0, 0), (2, 2), (2, 2)))

    sel = np.zeros((CENC, 4), np.float32)
    for p in range(CENC):
        sel[p, p % 4] = 1.0
    shmat = np.zeros((K_UP, W, LCH), np.float32)
    for s in range(K_UP):
        for w in range(W):
            shmat[s, w, w + s] = 1.0
    lsidx = np.full((LCH, CENC), -1, np.int16)
    for p in range(WP):
        for s in range(K_UP):
            w = p - s
            if 0 <= w < W:
                for ki in range(K_UP):
                    for u in range(4):
                        ry, rx = u // 2, u % 2
                        c = s * 20 + ki * 4 + u
                        lsidx[p, c] = ki * 256 + ry * 128 + 2 * w + rx
    comp_wT = (
        np.asarray(inputs["comp_w"], np.float32)[:, :, 0, 0].T.reshape(2, 128, CMID)
    )
    enc_wT = (
        np.asarray(inputs["enc_w"], np.float32)
        .reshape(CENC, CMID, 9)
        .transpose(2, 1, 0)
    )

    # bf16 pack: ident | y1mask(per-core) | shmat | encT | compT
    bfpk = np.zeros((128, BFPK), np.float32)
    o = 0
    bfpk[:, o : o + 128] = np.eye(128)
    o_mask = o = o + 128
    o += 660
    bfpk[0:W, o : o + K_UP * LCH] = shmat.transpose(1, 0, 2).reshape(W, K_UP * LCH)
    o += K_UP * LCH
    bfpk[0:CMID, o : o + 9 * CENC] = enc_wT.transpose(1, 0, 2).reshape(
        CMID, 9 * CENC
    )
    o += 9 * CENC
    bfpk[:, o : o + 2 * CMID] = comp_wT.transpose(1, 0, 2).reshape(128, 2 * CMID)

    # f32 pack: sel | selT | bn(comp) | bn(enc)
    f32pk = np.zeros((CENC, F32PK), np.float32)
    f32pk[:, 0:4] = sel
    f32pk[0:4, 4 : 4 + CENC] = sel.T
    for i, k in enumerate(("gamma", "beta", "mean", "var")):
        f32pk[0:CMID, 4 + CENC + i] = np.asarray(inputs[f"comp_{k}"], np.float32)
        f32pk[:, 4 + CENC + 4 + i] = np.asarray(inputs[f"enc_{k}"], np.float32)

    common = {
        "power_p": np.asarray(inputs["power_p"], np.float32),
        "f32pk": f32pk,
        "lsidx": lsidx,
    }
    in_maps = []
    for core in range(N_CORES):
        r0 = HS * core
        xh4 = np.ascontiguousarray(Xp[:, :, r0 : r0 + XROWS, :]).astype(bf16)
        mask = np.zeros((10, 66), np.float32)
        for rr in range(10):
            grow = r0 - 1 + rr
            if 0 <= grow < H:
                mask[rr, 1 : 1 + W] = 1.0
        bfpk_c = bfpk.copy()
        bfpk_c[0:CMID, o_mask : o_mask + 660] = mask.reshape(1, 660)
        m = dict(common)
        m["xh"] = xh4.reshape(B, 2, 128, XROWS * WP)
        m["xtin"] = np.ascontiguousarray(xh4.transpose(0, 2, 3, 1))
        m["bfpk"] = bfpk_c.astype(bf16)
        in_maps.append(m)
    return in_maps


def _run(inputs, trace=False):
    from concourse.bass_utils import run_bass_kernel_spmd

    if trace:
        import sys, os
        sys.path.insert(0, os.path.dirname(os.path.abspath(__file__)))
        import hookshim  # noqa: F401

    nc = _get_nc()
    in_maps = _make_in_maps(inputs)
    res = run_bass_kernel_spmd(
        nc, in_maps, core_ids=list(range(N_CORES)), trace=trace
    )
    out = np.concatenate([res.results[c]["out"] for c in range(N_CORES)], axis=2)
    return out, res


def kernel(**inputs):
    out, _ = _run(inputs, trace=False)
    return out


# revision 40
# speedup vs baseline: 19.3915x; 1.0363x over previous
"""CARAFE (content-aware upsample, power-normalized softmax) on 8 TRN2 cores.

Math (reference.py): X (2,256,64,64) ->
  conv1x1(256->64) + bn + relu -> conv3x3(64->100) + bn -> pixel_shuffle(2)
  -> W (2,25,128,128) -> softmax(clip(W)^p) over 25 taps
  out[b,c,y,x] = sum_{ki,kj} W[b,(ki,kj),y,x] * Xpad[b,c,y//2+ki-2,x//2+kj-2]

Strategy (pure data-parallel over h, 8 low-res rows / core):
  * conv1x1 / conv3x3 as bf16 GEMMs (channels on partitions).
  * softmax via ACT transcendentals; tap-sums via a 100x4 selection matmul;
    reciprocal on [4,512]; denominator broadcast back to 100 partitions via
    a 4x100 selection matmul (no DRAM bounce).
  * The per-pixel 25-tap weighted sum is a banded matmul per output row h:
    out[c,(ry,x)] = sum_p XT_r[p,c] * B_ki[p,(ry,x)] accumulated over ki,
    where B_ki[w+kj, ry*128+2w+rx] = Wnorm[(ki,kj,ry,rx), h, w].  B is built
    ON-CHIP: PE-transpose Wnorm rows -> 5 partition-shift matmuls (constant
    shift matrices) -> one GPSIMD local_scatter per (b,h) placing the
    diagonal bands (per-partition indices, zeros implicit).
  * XT_r strips come from PE transposes of the input rows.

kernel(**inputs) takes the FULL inputs and returns the FULL output.
"""

import numpy as np
import ml_dtypes

SCALE = 2
K_UP = 5
B, C, H, W = 2, 256, 64, 64
N_CORES = 8
HS = H // N_CORES            # 8 low-res rows per core
XROWS = HS + 4               # 12 rows (with +-2 halo)
WP = W + 4                   # 68 (w padded by 2 each side)
CMID, CENC = 64, 100
NSLOT = K_UP * 2 * W * SCALE // 2  # bts columns per ki = 256
BTN = K_UP * 256             # 1280 elems per bts row
LCH = 80                     # local_scatter channels (68 rounded up to 16x)

_STATE = {}


def _build_nc():
    import concourse.bass as bass
    import concourse.tile as tile
    from concourse import mybir
    from concourse.vector_clock import ScopedClock
    from concourse.tile_rust import add_dep_helper

    # --- workaround: this walrus build rejects >1 sync-wait on CTRL-class
    # instructions; split the Tile tail-drain waits into 1-wait NOPs. ---
    def patched_drain_and_barrier(self, tick_clock, wait_clock):
        maxw = 1
        carrier = self.nc.sync.nop()
        wait_clock.add_sem_waits(
            carrier.ins, ScopedClock({None: tick_clock.global_clock})
        )
        si = carrier.ins.sync_info
        waits = list(si.on_wait) if si is not None else []
        if len(waits) > maxw:
            si.on_wait = waits[:maxw]
            carrier.ins.sync_info = si
            rest = waits[maxw:]
            for i in range(0, len(rest), maxw):
                n = self.nc.sync.nop()
                n.ins.sync_info = mybir.SyncInfo(
                    on_wait=rest[i : i + maxw], on_update=[]
                )
        self.nc.sync.drain()
        self.nc.all_engine_barrier()
        assert self.sems is not None
        popped = self.nc._tile_sem_poison_stack.pop()
        assert popped is self._sem_poison
        self.nc.clear_and_free_semaphores(list(self.sems.allocated().values()))
        self.nc.all_engine_barrier()

    tile.TileContext._drain_and_barrier = patched_drain_and_barrier

    # --- workaround #2: the same walrus build accepts at most ONE sync wait
    # on ANY instruction.  Post-process the serialized BIR: hoist excess
    # waits onto single-wait NoOps inserted just before, on the same engine
    # (same program point, so semantics are unchanged). ---
    import orjson

    def _split_waits_json(raw: bytes) -> bytes:
        j = orjson.loads(raw)
        n = 0
        changed = False
        for fn in j["functions"]:
            for bb in fn["blocks"]:
                out = []
                for ins in bb["instructions"]:
                    si = ins.get("sync_info")
                    waits = si.get("on_wait") if si else None
                    if waits and len(waits) > 1:
                        changed = True
                        for wt in waits[:-1]:
                            n += 1
                            out.append(
                                {
                                    "debug": ins.get("debug", 0),
                                    "engine": ins["engine"],
                                    "ins": [],
                                    "outs": [],
                                    "name": f"WSPL-{n}",
                                    "opcode": "NoOp",
                                    "sync_info": {"on_update": [], "on_wait": [wt]},
                                }
                            )
                        si["on_wait"] = [waits[-1]]
                    out.append(ins)
                bb["instructions"] = out
        return orjson.dumps(j) if changed else raw

    if not getattr(bass.Bass.to_json_bytes, "_wait_split", False):
        _orig_tjb = bass.Bass.to_json_bytes

        def patched_to_json_bytes(self):
            return _split_waits_json(_orig_tjb(self))

        patched_to_json_bytes._wait_split = True
        bass.Bass.to_json_bytes = patched_to_json_bytes

    f32 = mybir.dt.float32
    bf16 = mybir.dt.bfloat16
    i16 = mybir.dt.int16
    AF = mybir.ActivationFunctionType

    nc = bass.Bass()

    # ---- parameters ----
    # Packed inputs (host-prepared layouts; see _make_in_maps):
    #  xh    [B, 2, 128, XROWS*WP]   bf16  image, channels on partitions
    #  xtin  [B, XROWS, WP, 256]     bf16  image pre-transposed (w on partitions)
    #  bfpk  [128, BFPK]             bf16  ident|mask|shmat|encT|compT
    #  f32pk [CENC, F32PK]           f32   sel|selT|bn params
    #  lsidx [LCH, CENC]             i16   local_scatter indices
    BFPK = 128 + 660 + K_UP * LCH + 9 * CENC + 2 * CMID
    F32PK = 4 + CENC + 4 + 4
    xh = nc.declare_dram_parameter("xh", [B, 2, 128, XROWS * WP], bf16, isOutput=False)
    xtin = nc.declare_dram_parameter(
        "xtin", [B, XROWS, WP, 256], bf16, isOutput=False
    )
    bfpk = nc.declare_dram_parameter("bfpk", [128, BFPK], bf16, isOutput=False)
    f32pk = nc.declare_dram_parameter("f32pk", [CENC, F32PK], f32, isOutput=False)
    p_in = nc.declare_dram_parameter("power_p", [1], f32, isOutput=False)
    lsidx = nc.declare_dram_parameter("lsidx", [LCH, CENC], i16, isOutput=False)

    out = nc.declare_dram_parameter(
        "out", [B, C, 2 * HS, 2 * W], f32, isOutput=True
    )
    # donated-zero scratch output (never read host-side)
    pscr = nc.declare_dram_parameter("pscr", [1], f32, isOutput=True)

    def dram_ap(param, offset, dims):
        return bass.AP(tensor=param, offset=offset, ap=[list(d) for d in dims])

    with tile.TileContext(nc) as tc:
        import contextlib

        ctx = contextlib.ExitStack()
        const = ctx.enter_context(tc.tile_pool(name="const", bufs=1))
        sm = ctx.enter_context(tc.tile_pool(name="sm", bufs=2))
        dp = ctx.enter_context(tc.tile_pool(name="dp", bufs=4))
        btp = ctx.enter_context(tc.tile_pool(name="btp", bufs=16))
        op = ctx.enter_context(tc.tile_pool(name="op", bufs=4))
        ps_big = ctx.enter_context(tc.tile_pool(name="ps_big", bufs=2, space="PSUM"))
        ps_bf = ctx.enter_context(tc.tile_pool(name="ps_bf", bufs=2, space="PSUM"))
        ps_sh = ctx.enter_context(tc.tile_pool(name="ps_sh", bufs=2, space="PSUM"))
        ps_e = ctx.enter_context(tc.tile_pool(name="ps_e", bufs=2, space="PSUM"))

        # ---- packed constants in SBUF (one DMA per dtype class) ----
        bf_sb = const.tile([128, BFPK], bf16, tag="bfpk")
        nc.sync.dma_start(out=bf_sb[:, :], in_=bfpk[:, :])
        f32_sb = const.tile([CENC, F32PK], f32, tag="f32pk")
        nc.scalar.dma_start(out=f32_sb[:, :], in_=f32pk[:, :])
        lsidx_sb = const.tile([LCH, CENC], i16, tag="lsidx")
        nc.scalar.dma_start(out=lsidx_sb[:, :], in_=lsidx[:, :])

        o_id = 0
        ident_sb = bf_sb[:, 0:128]
        o_id += 128
        mask_v = bf_sb[0:CMID, o_id : o_id + 660].rearrange(
            "p (a b) -> p a b", b=66
        )
        o_id += 660
        sh_all = bf_sb[0:W, o_id : o_id + K_UP * LCH].rearrange(
            "p (a b) -> p a b", b=LCH
        )
        o_id += K_UP * LCH
        enc_bf = []
        for j in range(9):
            enc_bf.append(bf_sb[0:CMID, o_id : o_id + CENC])
            o_id += CENC
        comp_bf = []
        for ct in range(2):
            comp_bf.append(bf_sb[:, o_id : o_id + CMID])
            o_id += CMID

        sel_sb = f32_sb[:, 0:4]
        selT_sb = f32_sb[0:4, 4 : 4 + CENC]

        # ---- batchnorm fold: inv = gamma/sqrt(var+eps), shift = beta-mean*inv
        def bn_fold(n, col0, tagp):
            g = f32_sb[0:n, col0 : col0 + 1]
            bt = f32_sb[0:n, col0 + 1 : col0 + 2]
            m = f32_sb[0:n, col0 + 2 : col0 + 3]
            v = f32_sb[0:n, col0 + 3 : col0 + 4]
            eps = const.tile([n, 1], f32, tag=f"{tagp}e")
            nc.vector.memset(eps[:, :], 1e-5)
            std = const.tile([n, 1], f32, tag=f"{tagp}s")
            nc.scalar.activation(std[:, :], v, AF.Sqrt, bias=eps[:, :])
            rstd = const.tile([n, 1], f32, tag=f"{tagp}r")
            nc.vector.reciprocal(rstd[:, :], std[:, :])
            inv = const.tile([n, 1], f32, tag=f"{tagp}i")
            nc.vector.tensor_mul(inv[:, :], g, rstd[:, :])
            tmp = const.tile([n, 1], f32, tag=f"{tagp}t")
            nc.vector.tensor_mul(tmp[:, :], m, inv[:, :])
            shift = const.tile([n, 1], f32, tag=f"{tagp}h")
            nc.vector.tensor_sub(shift[:, :], bt, tmp[:, :])
            return inv, shift

        inv1, shift1 = bn_fold(CMID, 4 + CENC, "bn1")
        inv2, shift2 = bn_fold(CENC, 4 + CENC + 4, "bn2")

        # ---- p = clip(power_p, 1e-5), broadcast to [100,1] via DRAM bounce
        p_sb = const.tile([1, 1], f32, tag="p")
        nc.sync.dma_start(out=p_sb[:, :], in_=dram_ap(p_in, 0, [[1, 1]]))
        nc.vector.tensor_scalar_max(p_sb[:, :], p_sb[:, :], 1e-5)
        p_wr = nc.sync.dma_start(out=dram_ap(pscr, 0, [[1, 1]]), in_=p_sb[:, :])
        pb_sb = const.tile([CENC, 1], f32, tag="pb")
        p_rd = nc.sync.dma_start(
            out=pb_sb[:, :], in_=dram_ap(pscr, 0, [[0, CENC], [1, 1]])
        )
        add_dep_helper(p_rd.ins, p_wr.ins, sync=True, reason="pscr RAW")

        # ---- X rows in SBUF (bf16, already padded on host) ----
        xbf = []
        for b in range(B):
            t = const.tile([128, 2, XROWS, WP], bf16, tag=f"xbf{b}")
            eng = nc.sync if b == 0 else nc.scalar
            eng.dma_start(
                out=t[:, :, :, :],
                in_=dram_ap(
                    xh,
                    b * 2 * 128 * XROWS * WP,
                    [
                        [XROWS * WP, 128],
                        [128 * XROWS * WP, 2],
                        [1, XROWS * WP],
                    ],
                ),
            )
            xbf.append(t)

        # ---- Y1 tiles (zeroed once; borders stay zero) ----
        y1 = []
        for b in range(B):
            t = const.tile([CMID, 10, 66], bf16, tag=f"y1_{b}")
            nc.vector.memset(t[:, :, :], 0.0)
            y1.append(t)

        # ---- X^T strips loaded pre-transposed from host ----
        xts_all = []
        for b in range(B):
            t = const.tile([WP, XROWS, 256], bf16, tag=f"xts{b}")
            eng = nc.scalar if b == 0 else nc.sync
            eng.dma_start(
                out=t[:, :, :],
                in_=dram_ap(
                    xtin,
                    b * XROWS * WP * 256,
                    [[256, WP], [WP * 256, XROWS], [1, 256]],
                ),
            )
            xts_all.append(t)

        en_sbs = {}
        bts_alls = {}

        def prep(b):
            # ===== conv1x1 + bn1 + relu =====
            for half in range(2):
                pcb = ps_big.tile([CENC, HS * W], f32, tag="big")
                pc = pcb[0:CMID, 0:320]
                for ct in range(2):
                    nc.tensor.matmul(
                        pc,
                        comp_bf[ct],
                        xbf[b][:, ct, 1 + 5 * half : 6 + 5 * half, 2 : 2 + W],
                        start=(ct == 0),
                        stop=(ct == 1),
                    )
                nc.scalar.activation(
                    y1[b][:, 5 * half : 5 * half + 5, 1 : 1 + W],
                    pc,
                    AF.Relu,
                    bias=shift1[:, :],
                    scale=inv1[:, :],
                )
            # zero out-of-image rows / padding cols
            nc.vector.tensor_mul(y1[b][:, :, :], y1[b][:, :, :], mask_v)

            # ===== conv3x3 + bn2 =====
            pc3 = ps_big.tile([CENC, HS * W], f32, tag="big")
            jj = 0
            for dy in (-1, 0, 1):
                for dx in (-1, 0, 1):
                    nc.tensor.matmul(
                        pc3[:, :],
                        enc_bf[jj],
                        y1[b][:, 1 + dy : 9 + dy, 1 + dx : 1 + dx + W],
                        start=(jj == 0),
                        stop=(jj == 8),
                    )
                    jj += 1
            w_sb = sm.tile([CENC, HS * W], f32, tag="w")
            nc.scalar.activation(
                w_sb[:, :], pc3[:, :], AF.Identity, bias=shift2[:, :], scale=inv2[:, :]
            )

            # ===== power + softmax numerator =====
            nc.vector.tensor_scalar_max(w_sb[:, :], w_sb[:, :], 1e-5)
            nc.scalar.activation(w_sb[:, :], w_sb[:, :], AF.Ln)
            nc.scalar.activation(w_sb[:, :], w_sb[:, :], AF.Exp, scale=pb_sb[:, :])
            e_sb = sm.tile([CENC, HS * W], f32, tag="e")
            nc.scalar.activation(e_sb[:, :], w_sb[:, :], AF.Exp)

            # ===== tap-sums, reciprocal, broadcast, normalize =====
            psb = ps_big.tile([CENC, HS * W], f32, tag="big")
            ps = psb[0:4, :]
            nc.tensor.matmul(ps, sel_sb, e_sb[:, :], start=True, stop=True)
            r4_sb = sm.tile([4, HS * W], f32, tag="r4")
            nc.vector.reciprocal_approx_fast(r4_sb[:, :], ps)
            rb_ps = ps_big.tile([CENC, HS * W], f32, tag="big")
            nc.tensor.matmul(
                rb_ps[:, :], selT_sb, r4_sb[:, :], start=True, stop=True
            )
            en_sb = const.tile([CENC, HS, W], bf16, tag=f"en{b}")
            nc.vector.tensor_mul(
                en_sb[:, :, :],
                e_sb[:, :].rearrange("p (a b) -> p a b", b=W),
                rb_ps[:, :].rearrange("p (a b) -> p a b", b=W),
            )
            en_sbs[b] = en_sb

        def band(b):
            # ===== banded-matrix build: transpose + shifts + local_scatter ===
            en_sb = en_sbs[b]
            bts_all = []
            tp_sbs = {}

            def emit_transpose(h):
                tpt = ps_bf.tile([W, CENC], bf16, tag="bf")
                tp_ps = tpt[:, :]
                nc.tensor.transpose(
                    tp_ps, en_sb[:, h, :], ident_sb[0:CENC, 0:CENC]
                )
                t = dp.tile([W, CENC], bf16, tag="tps")
                nc.vector.tensor_copy(t[:, :], tp_ps)
                tp_sbs[h] = t

            emit_transpose(0)
            for h in range(HS):
                if h + 1 < HS:
                    emit_transpose(h + 1)
                tp_sb = tp_sbs.pop(h)
                sh_ps = ps_sh.tile([LCH, CENC], f32, tag="sh")
                tp_v = tp_sb[:, :].rearrange("p (a b) -> p a b", b=20)
                for s in range(K_UP):
                    nc.tensor.matmul(
                        sh_ps[:, 20 * s : 20 * (s + 1)],
                        sh_all[:, s, :],
                        tp_v[:, :, 4 * s : 4 * (s + 1)],
                        start=True,
                        stop=True,
                    )
                data_sb = dp.tile([LCH, CENC], bf16, tag="data")
                nc.vector.tensor_copy(data_sb[:, :], sh_ps[:, :])
                bts = btp.tile([LCH, BTN], bf16, tag="bts")
                nc.gpsimd.local_scatter(
                    out_ap=bts[:, :],
                    data_ap=data_sb[:, :],
                    idxs_ap=lsidx_sb[:, :],
                    channels=LCH,
                    num_elems=BTN,
                    num_idxs=CENC,
                )
                bts_all.append(bts)
            bts_alls[b] = bts_all

        def eins(b):
            # ===== banded einsum (h-pairs batched into one out DMA) =====
            bts_all = bts_alls[b]
            for hp in range(HS // 2):
                for ct in range(2):
                    o_sb = op.tile([128, 512], f32, tag="osb")
                    for hh in range(2):
                        h = 2 * hp + hh
                        bts = bts_all[h]
                        pe = ps_e.tile([128, 256], f32, tag="pe")
                        for ki in range(K_UP):
                            nc.tensor.matmul(
                                pe[:, :],
                                xts_all[b][:, h + ki, ct * 128 : (ct + 1) * 128],
                                bts[0:WP, ki * 256 : (ki + 1) * 256],
                                start=(ki == 0),
                                stop=(ki == K_UP - 1),
                            )
                        dst = o_sb[:, 256 * hh : 256 * (hh + 1)]
                        if (hh + ct) % 2 == 0:
                            nc.scalar.activation(dst, pe[:, :], AF.Identity)
                        else:
                            nc.vector.tensor_copy(dst, pe[:, :])
                    oeng = nc.sync if ct == 0 else nc.scalar
                    oeng.dma_start(
                        out=dram_ap(
                            out,
                            b * C * 2 * HS * 2 * W
                            + ct * 128 * 2 * HS * 2 * W
                            + 4 * hp * 2 * W,
                            [[2 * HS * 2 * W, 128], [1, 512]],
                        ),
                        in_=o_sb[:, :],
                    )

        prep(0)
        band(0)
        prep(1)
        eins(0)
        band(1)
        eins(1)

        ctx.close()

    # ---- Bacc-style finishing passes: library loads + ISA assembly ----
    from concourse.library_config import all_libraries, standard
    import bass_rust as _bass_rust

    lib_mask = {}
    for lib in all_libraries:
        for it in lib.instructions:
            lib_mask[it] = lib_mask.get(it, 0) | (1 << lib.index)
    _bass_rust.insert_library_loads(nc, lib_mask, len(all_libraries), standard.index)
    mybir.codegen_inst_isa_subclasses(nc)

    return nc


def _get_nc():
    if "nc" not in _STATE:
        _STATE["nc"] = _build_nc()
    return _STATE["nc"]


def _make_in_maps(inputs):
    bf16 = ml_dtypes.bfloat16
    BFPK = 128 + 660 + K_UP * LCH + 9 * CENC + 2 * CMID
    F32PK = 4 + CENC + 4 + 4
    X = np.asarray(inputs["X"], dtype=np.float32)
    Xp = np.pad(X, ((0, 0), (0, 0), (2, 2), (2, 2)))

    sel = np.zeros((CENC, 4), np.float32)
    for p in range(CENC):
        sel[p, p % 4] = 1.0
    shmat = np.zeros((K_UP, W, LCH), np.float32)
    for s in range(K_UP):
        for w in range(W):
            shmat[s, w, w + s] = 1.0
    lsidx = np.full((LCH, CENC), -1, np.int16)
    for p in range(WP):
        for s in range(K_UP):
            w = p - s
            if 0 <= w < W:
                for ki in range(K_UP):
                    for u in range(4):
                        ry, rx = u // 2, u % 2
                        c = s * 20 + ki * 4 + u
                        lsidx[p, c] = ki * 256 + ry * 128 + 2 * w + rx
    comp_wT = (
        np.asarray(inputs["comp_w"], np.float32)[:, :, 0, 0].T.reshape(2, 128, CMID)
    )
    enc_wT = (
        np.asarray(inputs["enc_w"], np.float32)
        .reshape(CENC, CMID, 9)
        .transpose(2, 1, 0)
    )

    # bf16 pack: ident | y1mask(per-core) | shmat | encT | compT
    bfpk = np.zeros((128, BFPK), np.float32)
    o = 0
    bfpk[:, o : o + 128] = np.eye(128)
    o_mask = o = o + 128
    o += 660
    bfpk[0:W, o : o + K_UP * LCH] = shmat.transpose(1, 0, 2).reshape(W, K_UP * LCH)
    o += K_UP * LCH
    bfpk[0:CMID, o : o + 9 * CENC] = enc_wT.transpose(1, 0, 2).reshape(
        CMID, 9 * CENC
    )
    o += 9 * CENC
    bfpk[:, o : o + 2 * CMID] = comp_wT.transpose(1, 0, 2).reshape(128, 2 * CMID)

    # f32 pack: sel | selT | bn(comp) | bn(enc)
    f32pk = np.zeros((CENC, F32PK), np.float32)
    f32pk[:, 0:4] = sel
    f32pk[0:4, 4 : 4 + CENC] = sel.T
    for i, k in enumerate(("gamma", "beta", "mean", "var")):
        f32pk[0:CMID, 4 + CENC + i] = np.asarray(inputs[f"comp_{k}"], np.float32)
        f32pk[:, 4 + CENC + 4 + i] = np.asarray(inputs[f"enc_{k}"], np.float32)

    common = {
        "power_p": np.asarray(inputs["power_p"], np.float32),
        "f32pk": f32pk,
        "lsidx": lsidx,
    }
    in_maps = []
    for core in range(N_CORES):
        r0 = HS * core
        xh4 = np.ascontiguousarray(Xp[:, :, r0 : r0 + XROWS, :]).astype(bf16)
        mask = np.zeros((10, 66), np.float32)
        for rr in range(10):
            grow = r0 - 1 + rr
            if 0 <= grow < H:
                mask[rr, 1 : 1 + W] = 1.0
        bfpk_c = bfpk.copy()
        bfpk_c[0:CMID, o_mask : o_mask + 660] = mask.reshape(1, 660)
        m = dict(common)
        m["xh"] = xh4.reshape(B, 2, 128, XROWS * WP)
        m["xtin"] = np.ascontiguousarray(xh4.transpose(0, 2, 3, 1))
        m["bfpk"] = bfpk_c.astype(bf16)
        in_maps.append(m)
    return in_maps


def _run(inputs, trace=False):
    from concourse.bass_utils import run_bass_kernel_spmd

    if trace:
        import sys, os
        sys.path.insert(0, os.path.dirname(os.path.abspath(__file__)))
        import hookshim  # noqa: F401

    nc = _get_nc()
    in_maps = _make_in_maps(inputs)
    res = run_bass_kernel_spmd(
        nc, in_maps, core_ids=list(range(N_CORES)), trace=trace
    )
    out = np.concatenate([res.results[c]["out"] for c in range(N_CORES)], axis=2)
    return out, res


def kernel(**inputs):
    out, _ = _run(inputs, trace=False)
    return out


# revision 42
# speedup vs baseline: 20.4501x; 1.0546x over previous
"""CARAFE (content-aware upsample, power-normalized softmax) on 8 TRN2 cores.

Math (reference.py): X (2,256,64,64) ->
  conv1x1(256->64) + bn + relu -> conv3x3(64->100) + bn -> pixel_shuffle(2)
  -> W (2,25,128,128) -> softmax(clip(W)^p) over 25 taps
  out[b,c,y,x] = sum_{ki,kj} W[b,(ki,kj),y,x] * Xpad[b,c,y//2+ki-2,x//2+kj-2]

Strategy (pure data-parallel over h, 8 low-res rows / core):
  * conv1x1 / conv3x3 as bf16 GEMMs (channels on partitions).
  * softmax via ACT transcendentals; tap-sums via a 100x4 selection matmul;
    reciprocal on [4,512]; denominator broadcast back to 100 partitions via
    a 4x100 selection matmul (no DRAM bounce).
  * The per-pixel 25-tap weighted sum is a banded matmul per output row h:
    out[c,(ry,x)] = sum_p XT_r[p,c] * B_ki[p,(ry,x)] accumulated over ki,
    where B_ki[w+kj, ry*128+2w+rx] = Wnorm[(ki,kj,ry,rx), h, w].  B is built
    ON-CHIP: PE-transpose Wnorm rows -> 5 partition-shift matmuls (constant
    shift matrices) -> one GPSIMD local_scatter per (b,h) placing the
    diagonal bands (per-partition indices, zeros implicit).
  * XT_r strips come from PE transposes of the input rows.

kernel(**inputs) takes the FULL inputs and returns the FULL output.
"""

import numpy as np
import ml_dtypes

SCALE = 2
K_UP = 5
B, C, H, W = 2, 256, 64, 64
N_CORES = 8
HS = H // N_CORES            # 8 low-res rows per core
XROWS = HS + 4               # 12 rows (with +-2 halo)
WP = W + 4                   # 68 (w padded by 2 each side)
CMID, CENC = 64, 100
NSLOT = K_UP * 2 * W * SCALE // 2  # bts columns per ki = 256
BTN = K_UP * 256             # 1280 elems per bts row
LCH = 80                     # local_scatter channels (68 rounded up to 16x)

_STATE = {}


def _build_nc():
    import concourse.bass as bass
    import concourse.tile as tile
    from concourse import mybir
    from concourse.vector_clock import ScopedClock
    from concourse.tile_rust import add_dep_helper

    # --- workaround: this walrus build rejects >1 sync-wait on CTRL-class
    # instructions; split the Tile tail-drain waits into 1-wait NOPs. ---
    def patched_drain_and_barrier(self, tick_clock, wait_clock):
        maxw = 1
        carrier = self.nc.sync.nop()
        wait_clock.add_sem_waits(
            carrier.ins, ScopedClock({None: tick_clock.global_clock})
        )
        si = carrier.ins.sync_info
        waits = list(si.on_wait) if si is not None else []
        if len(waits) > maxw:
            si.on_wait = waits[:maxw]
            carrier.ins.sync_info = si
            rest = waits[maxw:]
            for i in range(0, len(rest), maxw):
                n = self.nc.sync.nop()
                n.ins.sync_info = mybir.SyncInfo(
                    on_wait=rest[i : i + maxw], on_update=[]
                )
        self.nc.sync.drain()
        self.nc.all_engine_barrier()
        assert self.sems is not None
        popped = self.nc._tile_sem_poison_stack.pop()
        assert popped is self._sem_poison
        self.nc.clear_and_free_semaphores(list(self.sems.allocated().values()))
        self.nc.all_engine_barrier()

    tile.TileContext._drain_and_barrier = patched_drain_and_barrier

    # --- workaround #2: the same walrus build accepts at most ONE sync wait
    # on ANY instruction.  Post-process the serialized BIR: hoist excess
    # waits onto single-wait NoOps inserted just before, on the same engine
    # (same program point, so semantics are unchanged). ---
    import orjson

    def _split_waits_json(raw: bytes) -> bytes:
        j = orjson.loads(raw)
        n = 0
        changed = False
        for fn in j["functions"]:
            for bb in fn["blocks"]:
                out = []
                for ins in bb["instructions"]:
                    si = ins.get("sync_info")
                    waits = si.get("on_wait") if si else None
                    if waits and len(waits) > 1:
                        changed = True
                        for wt in waits[:-1]:
                            n += 1
                            out.append(
                                {
                                    "debug": ins.get("debug", 0),
                                    "engine": ins["engine"],
                                    "ins": [],
                                    "outs": [],
                                    "name": f"WSPL-{n}",
                                    "opcode": "NoOp",
                                    "sync_info": {"on_update": [], "on_wait": [wt]},
                                }
                            )
                        si["on_wait"] = [waits[-1]]
                    out.append(ins)
                bb["instructions"] = out
        return orjson.dumps(j) if changed else raw

    if not getattr(bass.Bass.to_json_bytes, "_wait_split", False):
        _orig_tjb = bass.Bass.to_json_bytes

        def patched_to_json_bytes(self):
            return _split_waits_json(_orig_tjb(self))

        patched_to_json_bytes._wait_split = True
        bass.Bass.to_json_bytes = patched_to_json_bytes

    f32 = mybir.dt.float32
    bf16 = mybir.dt.bfloat16
    i16 = mybir.dt.int16
    AF = mybir.ActivationFunctionType

    nc = bass.Bass()

    # ---- parameters ----
    # Packed inputs (host-prepared layouts; see _make_in_maps):
    #  xh    [B, 2, 128, XROWS*WP]   bf16  image, channels on partitions
    #  xtin  [B, XROWS, WP, 256]     bf16  image pre-transposed (w on partitions)
    #  bfpk  [128, BFPK]             bf16  ident|mask|shmat|encT|compT
    #  f32pk [CENC, F32PK]           f32   sel|selT|bn params
    #  lsidx [LCH, CENC]             i16   local_scatter indices
    BFPK = 128 + 660 + K_UP * LCH + 9 * CENC + 2 * CMID
    F32PK = 4 + CENC + 4 + 4
    xh = nc.declare_dram_parameter("xh", [B, 2, 128, XROWS * WP], bf16, isOutput=False)
    xtin = nc.declare_dram_parameter(
        "xtin", [B, XROWS, WP, 256], bf16, isOutput=False
    )
    bfpk = nc.declare_dram_parameter("bfpk", [128, BFPK], bf16, isOutput=False)
    f32pk = nc.declare_dram_parameter("f32pk", [CENC, F32PK], f32, isOutput=False)
    p_in = nc.declare_dram_parameter("power_p", [1], f32, isOutput=False)
    lsidx = nc.declare_dram_parameter("lsidx", [LCH, CENC], i16, isOutput=False)

    out = nc.declare_dram_parameter(
        "out", [B, C, 2 * HS, 2 * W], f32, isOutput=True
    )
    # donated-zero scratch output (never read host-side)
    pscr = nc.declare_dram_parameter("pscr", [1], f32, isOutput=True)

    def dram_ap(param, offset, dims):
        return bass.AP(tensor=param, offset=offset, ap=[list(d) for d in dims])

    with tile.TileContext(nc) as tc:
        import contextlib

        ctx = contextlib.ExitStack()
        const = ctx.enter_context(tc.tile_pool(name="const", bufs=1))
        sm = ctx.enter_context(tc.tile_pool(name="sm", bufs=2))
        dp = ctx.enter_context(tc.tile_pool(name="dp", bufs=4))
        btp = ctx.enter_context(tc.tile_pool(name="btp", bufs=16))
        op = ctx.enter_context(tc.tile_pool(name="op", bufs=4))
        ps_big = ctx.enter_context(tc.tile_pool(name="ps_big", bufs=2, space="PSUM"))
        ps_bf = ctx.enter_context(tc.tile_pool(name="ps_bf", bufs=2, space="PSUM"))
        ps_sh = ctx.enter_context(tc.tile_pool(name="ps_sh", bufs=2, space="PSUM"))
        ps_e = ctx.enter_context(tc.tile_pool(name="ps_e", bufs=2, space="PSUM"))

        # ---- packed constants in SBUF (one DMA per dtype class) ----
        bf_sb = const.tile([128, BFPK], bf16, tag="bfpk")
        nc.sync.dma_start(out=bf_sb[:, :], in_=bfpk[:, :])
        f32_sb = const.tile([CENC, F32PK], f32, tag="f32pk")
        nc.scalar.dma_start(out=f32_sb[:, :], in_=f32pk[:, :])
        lsidx_sb = const.tile([LCH, CENC], i16, tag="lsidx")
        nc.scalar.dma_start(out=lsidx_sb[:, :], in_=lsidx[:, :])

        o_id = 0
        ident_sb = bf_sb[:, 0:128]
        o_id += 128
        mask_v = bf_sb[0:CMID, o_id : o_id + 660].rearrange(
            "p (a b) -> p a b", b=66
        )
        o_id += 660
        sh_all = bf_sb[0:W, o_id : o_id + K_UP * LCH].rearrange(
            "p (a b) -> p a b", b=LCH
        )
        o_id += K_UP * LCH
        enc_bf = []
        for j in range(9):
            enc_bf.append(bf_sb[0:CMID, o_id : o_id + CENC])
            o_id += CENC
        comp_bf = []
        for ct in range(2):
            comp_bf.append(bf_sb[:, o_id : o_id + CMID])
            o_id += CMID

        sel_sb = f32_sb[:, 0:4]
        selT_sb = f32_sb[0:4, 4 : 4 + CENC]

        # ---- batchnorm fold: inv = gamma/sqrt(var+eps), shift = beta-mean*inv
        def bn_fold(n, col0, tagp):
            g = f32_sb[0:n, col0 : col0 + 1]
            bt = f32_sb[0:n, col0 + 1 : col0 + 2]
            m = f32_sb[0:n, col0 + 2 : col0 + 3]
            v = f32_sb[0:n, col0 + 3 : col0 + 4]
            eps = const.tile([n, 1], f32, tag=f"{tagp}e")
            nc.vector.memset(eps[:, :], 1e-5)
            std = const.tile([n, 1], f32, tag=f"{tagp}s")
            nc.scalar.activation(std[:, :], v, AF.Sqrt, bias=eps[:, :])
            rstd = const.tile([n, 1], f32, tag=f"{tagp}r")
            nc.vector.reciprocal(rstd[:, :], std[:, :])
            inv = const.tile([n, 1], f32, tag=f"{tagp}i")
            nc.vector.tensor_mul(inv[:, :], g, rstd[:, :])
            tmp = const.tile([n, 1], f32, tag=f"{tagp}t")
            nc.vector.tensor_mul(tmp[:, :], m, inv[:, :])
            shift = const.tile([n, 1], f32, tag=f"{tagp}h")
            nc.vector.tensor_sub(shift[:, :], bt, tmp[:, :])
            return inv, shift

        inv1, shift1 = bn_fold(CMID, 4 + CENC, "bn1")
        inv2, shift2 = bn_fold(CENC, 4 + CENC + 4, "bn2")

        # ---- p = clip(power_p, 1e-5), broadcast to [100,1] via DRAM bounce
        p_sb = const.tile([1, 1], f32, tag="p")
        nc.sync.dma_start(out=p_sb[:, :], in_=dram_ap(p_in, 0, [[1, 1]]))
        nc.vector.tensor_scalar_max(p_sb[:, :], p_sb[:, :], 1e-5)
        p_wr = nc.sync.dma_start(out=dram_ap(pscr, 0, [[1, 1]]), in_=p_sb[:, :])
        pb_sb = const.tile([CENC, 1], f32, tag="pb")
        p_rd = nc.sync.dma_start(
            out=pb_sb[:, :], in_=dram_ap(pscr, 0, [[0, CENC], [1, 1]])
        )
        add_dep_helper(p_rd.ins, p_wr.ins, sync=True, reason="pscr RAW")

        # ---- X rows in SBUF (bf16, already padded on host) ----
        xbf = []
        for b in range(B):
            t = const.tile([128, 2, XROWS, WP], bf16, tag=f"xbf{b}")
            eng = nc.sync if b == 0 else nc.scalar
            eng.dma_start(
                out=t[:, :, :, :],
                in_=dram_ap(
                    xh,
                    b * 2 * 128 * XROWS * WP,
                    [
                        [XROWS * WP, 128],
                        [128 * XROWS * WP, 2],
                        [1, XROWS * WP],
                    ],
                ),
            )
            xbf.append(t)

        # ---- Y1 tiles (zeroed once; borders stay zero) ----
        y1 = []
        for b in range(B):
            t = const.tile([CMID, 10, 66], bf16, tag=f"y1_{b}")
            nc.vector.memset(t[:, :, :], 0.0)
            y1.append(t)

        # ---- X^T strips loaded pre-transposed from host ----
        xts_all = []
        for b in range(B):
            t = const.tile([WP, XROWS, 256], bf16, tag=f"xts{b}")
            eng = nc.scalar if b == 0 else nc.sync
            eng.dma_start(
                out=t[:, :, :],
                in_=dram_ap(
                    xtin,
                    b * XROWS * WP * 256,
                    [[256, WP], [WP * 256, XROWS], [1, 256]],
                ),
            )
            xts_all.append(t)

        en_sbs = {}
        bts_alls = {}

        def prep(b):
            # ===== conv1x1 + bn1 + relu =====
            for half in range(2):
                pcb = ps_big.tile([CENC, HS * W], f32, tag="big")
                pc = pcb[0:CMID, 0:320]
                for ct in range(2):
                    nc.tensor.matmul(
                        pc,
                        comp_bf[ct],
                        xbf[b][:, ct, 1 + 5 * half : 6 + 5 * half, 2 : 2 + W],
                        start=(ct == 0),
                        stop=(ct == 1),
                    )
                nc.scalar.activation(
                    y1[b][:, 5 * half : 5 * half + 5, 1 : 1 + W],
                    pc,
                    AF.Relu,
                    bias=shift1[:, :],
                    scale=inv1[:, :],
                )
            # zero out-of-image rows / padding cols
            nc.vector.tensor_mul(y1[b][:, :, :], y1[b][:, :, :], mask_v)

            # ===== conv3x3 + bn2 =====
            pc3 = ps_big.tile([CENC, HS * W], f32, tag="big")
            jj = 0
            for dy in (-1, 0, 1):
                for dx in (-1, 0, 1):
                    nc.tensor.matmul(
                        pc3[:, :],
                        enc_bf[jj],
                        y1[b][:, 1 + dy : 9 + dy, 1 + dx : 1 + dx + W],
                        start=(jj == 0),
                        stop=(jj == 8),
                    )
                    jj += 1
            w_sb = sm.tile([CENC, HS * W], f32, tag="w")
            nc.scalar.activation(
                w_sb[:, :], pc3[:, :], AF.Identity, bias=shift2[:, :], scale=inv2[:, :]
            )

            # ===== power + softmax numerator =====
            nc.vector.tensor_scalar_max(w_sb[:, :], w_sb[:, :], 1e-5)
            nc.scalar.activation(w_sb[:, :], w_sb[:, :], AF.Ln)
            nc.scalar.activation(w_sb[:, :], w_sb[:, :], AF.Exp, scale=pb_sb[:, :])
            e_sb = sm.tile([CENC, HS * W], f32, tag="e")
            nc.scalar.activation(e_sb[:, :], w_sb[:, :], AF.Exp)

            # ===== tap-sums, reciprocal, broadcast, normalize =====
            psb = ps_big.tile([CENC, HS * W], f32, tag="big")
            ps = psb[0:4, :]
            nc.tensor.matmul(ps, sel_sb, e_sb[:, :], start=True, stop=True)
            r4_sb = sm.tile([4, HS * W], f32, tag="r4")
            nc.vector.reciprocal_approx_fast(r4_sb[:, :], ps)
            rb_ps = ps_big.tile([CENC, HS * W], f32, tag="big")
            nc.tensor.matmul(
                rb_ps[:, :], selT_sb, r4_sb[:, :], start=True, stop=True
            )
            en_sb = const.tile([CENC, HS, W], bf16, tag=f"en{b}")
            nc.vector.tensor_mul(
                en_sb[:, :, :],
                e_sb[:, :].rearrange("p (a b) -> p a b", b=W),
                rb_ps[:, :].rearrange("p (a b) -> p a b", b=W),
            )
            en_sbs[b] = en_sb

        def band(b):
            # ===== banded-matrix build (h-pairs): 2 transposes into one psum,
            # 5 paired shift matmuls, 2 data copies, 2 local_scatters into one
            # [LCH, 2*BTN] tile =====
            en_sb = en_sbs[b]
            bts_all = []
            for hp in range(HS // 2):
                tpt = ps_bf.tile([W, 2 * CENC], bf16, tag="bf")
                for hh in range(2):
                    nc.tensor.transpose(
                        tpt[:, CENC * hh : CENC * (hh + 1)],
                        en_sb[:, 2 * hp + hh, :],
                        ident_sb[0:CENC, 0:CENC],
                    )
                tp_pair = dp.tile([W, 2 * CENC], bf16, tag="tps")
                nc.vector.tensor_copy(tp_pair[:, :], tpt[:, :])
                # cols of tp_pair: hh*100 + ki*20 + kj*4 + u
                tp_v = tp_pair[:, :].rearrange(
                    "p (hh ki r) -> p hh ki r", hh=2, r=20
                )
                sh_ps = ps_sh.tile([LCH, 2 * CENC], f32, tag="sh")
                sh_v = sh_ps[:, :].rearrange("p (hh r) -> p hh r", hh=2)
                for s in range(K_UP):
                    # out cols (hh, 20) at base s*20; rhs (hh, ki, u) base s*4
                    nc.tensor.matmul(
                        sh_v[:, :, 20 * s : 20 * (s + 1)],
                        sh_all[:, s, :],
                        tp_v[:, :, :, 4 * s : 4 * (s + 1)],
                        start=True,
                        stop=True,
                    )
                btsp = btp.tile([LCH, 2 * BTN], bf16, tag="bts")
                for hh in range(2):
                    data_sb = dp.tile([LCH, CENC], bf16, tag="data")
                    nc.vector.tensor_copy(
                        data_sb[:, :], sh_ps[:, CENC * hh : CENC * (hh + 1)]
                    )
                    nc.gpsimd.local_scatter(
                        out_ap=btsp[:, BTN * hh : BTN * (hh + 1)],
                        data_ap=data_sb[:, :],
                        idxs_ap=lsidx_sb[:, :],
                        channels=LCH,
                        num_elems=BTN,
                        num_idxs=CENC,
                    )
                bts_all.append(btsp)
            bts_alls[b] = bts_all

        def eins(b):
            # ===== banded einsum: h-pair-fused matmuls (6 per pair per ct),
            # one out DMA per (pair, ct) =====
            bts_all = bts_alls[b]
            for hp in range(HS // 2):
                ha = 2 * hp
                btsp = bts_all[hp]
                for ct in range(2):
                    pe = ps_e.tile([128, 512], f32, tag="pe")
                    cs = ct * 128
                    # fused strips jj=1..4: regions (ha@ki=jj, hb@ki=jj-1)
                    base_ap = btsp[0:WP, 0:256]
                    for jj in range(1, K_UP):
                        rhs = bass.AP(
                            tensor=base_ap.tensor,
                            offset=base_ap.offset + jj * 256,
                            ap=[list(base_ap.ap[0]), [BTN - 256, 2], [1, 256]],
                        )
                        nc.tensor.matmul(
                            pe[:, :],
                            xts_all[b][:, ha + jj, cs : cs + 128],
                            rhs,
                            start=(jj == 1),
                            stop=False,
                        )
                    # single strip jj=0 -> region A only
                    nc.tensor.matmul(
                        pe[:, 0:256],
                        xts_all[b][:, ha, cs : cs + 128],
                        btsp[0:WP, 0:256],
                        start=False,
                        stop=True,
                    )
                    # single strip jj=5 -> region B only
                    nc.tensor.matmul(
                        pe[:, 256:512],
                        xts_all[b][:, ha + 5, cs : cs + 128],
                        btsp[0:WP, BTN + 4 * 256 : BTN + 5 * 256],
                        start=False,
                        stop=True,
                    )
                    o_sb = op.tile([128, 512], f32, tag="osb")
                    if ct == 0:
                        nc.scalar.activation(o_sb[:, :], pe[:, :], AF.Identity)
                    else:
                        nc.vector.tensor_copy(o_sb[:, :], pe[:, :])
                    oeng = nc.sync if ct == 0 else nc.scalar
                    oeng.dma_start(
                        out=dram_ap(
                            out,
                            b * C * 2 * HS * 2 * W
                            + ct * 128 * 2 * HS * 2 * W
                            + 4 * hp * 2 * W,
                            [[2 * HS * 2 * W, 128], [1, 512]],
                        ),
                        in_=o_sb[:, :],
                    )

        prep(0)
        band(0)
        prep(1)
        eins(0)
        band(1)
        eins(1)

        ctx.close()

    # ---- Bacc-style finishing passes: library loads + ISA assembly ----
    from concourse.library_config import all_libraries, standard
    import bass_rust as _bass_rust

    lib_mask = {}
    for lib in all_libraries:
        for it in lib.instructions:
            lib_mask[it] = lib_mask.get(it, 0) | (1 << lib.index)
    _bass_rust.insert_library_loads(nc, lib_mask, len(all_libraries), standard.index)
    mybir.codegen_inst_isa_subclasses(nc)

    return nc


def _get_nc():
    if "nc" not in _STATE:
        _STATE["nc"] = _build_nc()
    return _STATE["nc"]


def _make_in_maps(inputs):
    bf16 = ml_dtypes.bfloat16
    BFPK = 128 + 660 + K_UP * LCH + 9 * CENC + 2 * CMID
    F32PK = 4 + CENC + 4 + 4
    X = np.asarray(inputs["X"], dtype=np.float32)
    Xp = np.pad(X, ((0, 0), (0, 0), (2, 2), (2, 2)))

    sel = np.zeros((CENC, 4), np.float32)
    for p in range(CENC):
        sel[p, p % 4] = 1.0
    shmat = np.zeros((K_UP, W, LCH), np.float32)
    for s in range(K_UP):
        for w in range(W):
            shmat[s, w, w + s] = 1.0
    lsidx = np.full((LCH, CENC), -1, np.int16)
    for p in range(WP):
        for s in range(K_UP):
            w = p - s
            if 0 <= w < W:
                for ki in range(K_UP):
                    for u in range(4):
                        ry, rx = u // 2, u % 2
                        c = s * 20 + ki * 4 + u
                        lsidx[p, c] = ki * 256 + ry * 128 + 2 * w + rx
    comp_wT = (
        np.asarray(inputs["comp_w"], np.float32)[:, :, 0, 0].T.reshape(2, 128, CMID)
    )
    enc_wT = (
        np.asarray(inputs["enc_w"], np.float32)
        .reshape(CENC, CMID, 9)
        .transpose(2, 1, 0)
    )

    # bf16 pack: ident | y1mask(per-core) | shmat | encT | compT
    bfpk = np.zeros((128, BFPK), np.float32)
    o = 0
    bfpk[:, o : o + 128] = np.eye(128)
    o_mask = o = o + 128
    o += 660
    bfpk[0:W, o : o + K_UP * LCH] = shmat.transpose(1, 0, 2).reshape(W, K_UP * LCH)
    o += K_UP * LCH
    bfpk[0:CMID, o : o + 9 * CENC] = enc_wT.transpose(1, 0, 2).reshape(
        CMID, 9 * CENC
    )
    o += 9 * CENC
    bfpk[:, o : o + 2 * CMID] = comp_wT.transpose(1, 0, 2).reshape(128, 2 * CMID)

    # f32 pack: sel | selT | bn(comp) | bn(enc)
    f32pk = np.zeros((CENC, F32PK), np.float32)
    f32pk[:, 0:4] = sel
    f32pk[0:4, 4 : 4 + CENC] = sel.T
    for i, k in enumerate(("gamma", "beta", "mean", "var")):
        f32pk[0:CMID, 4 + CENC + i] = np.asarray(inputs[f"comp_{k}"], np.float32)
        f32pk[:, 4 + CENC + 4 + i] = np.asarray(inputs[f"enc_{k}"], np.float32)

    common = {
        "power_p": np.asarray(inputs["power_p"], np.float32),
        "f32pk": f32pk,
        "lsidx": lsidx,
    }
    in_maps = []
    for core in range(N_CORES):
        r0 = HS * core
        xh4 = np.ascontiguousarray(Xp[:, :, r0 : r0 + XROWS, :]).astype(bf16)
        mask = np.zeros((10, 66), np.float32)
        for rr in range(10):
            grow = r0 - 1 + rr
            if 0 <= grow < H:
                mask[rr, 1 : 1 + W] = 1.0
        bfpk_c = bfpk.copy()
        bfpk_c[0:CMID, o_mask : o_mask + 660] = mask.reshape(1, 660)
        m = dict(common)
        m["xh"] = xh4.reshape(B, 2, 128, XROWS * WP)
        m["xtin"] = np.ascontiguousarray(xh4.transpose(0, 2, 3, 1))
        m["bfpk"] = bfpk_c.astype(bf16)
        in_maps.append(m)
    return in_maps


def _run(inputs, trace=False):
    from concourse.bass_utils import run_bass_kernel_spmd

    if trace:
        import sys, os
        sys.path.insert(0, os.path.dirname(os.path.abspath(__file__)))
        import hookshim  # noqa: F401

    nc = _get_nc()
    in_maps = _make_in_maps(inputs)
    res = run_bass_kernel_spmd(
        nc, in_maps, core_ids=list(range(N_CORES)), trace=trace
    )
    out = np.concatenate([res.results[c]["out"] for c in range(N_CORES)], axis=2)
    return out, res


def kernel(**inputs):
    out, _ = _run(inputs, trace=False)
    return out
